# revision 1
# baseline (speedup 1.0000x reference)
"""Trainium2 Bass kernel for Brain3DQTUNNetwork (gnn_message_passing).

The "sparse" graph is a fixed Manhattan-radius-2 stencil on a 64^3 grid
(24 offsets).  Weights are stored dense per offset slot, keyed by the
DESTINATION (col) index: W[k][j] = w(edge j-d_k -> j), 0 for invalid
edges.  The per-step segment_sum SpMV becomes 24 shifted elementwise
multiply-accumulates, and the STDP update becomes
    w = clip(w*(1-WDECAY) + prev * q_shift,  0, 1),   q(o) = 0.015*o - 0.005.
Invalid slots self-heal to 0 every step (q reads 0 / q(0) < 0 there and
the clip floors at 0).

Layout: partition axis = x (64).  Free axis = (y_local + halo, z + pad):
12*68 = 816.  y/z shifts are free-dim AP offsets; x shifts are realized
as 4 SBUF->SBUF DMA partition-shifted copies (engine APs must start at a
32-aligned partition, DMA has no such constraint).

Sharding: 8 y-slabs of 8 y-planes each.  All state (weights, v, prev)
stays SBUF-resident for all 50 steps; per-step cross-core traffic is a
single 8-rank AllGather of the 2-row boundary strips, with neighbor
extraction via partition-id-driven dynamic DMA.
"""

import os
import sys

sys.path.insert(0, "/opt/trn_rl_repo")

import numpy as np

import concourse.bass as bass
import concourse.bacc as bacc
import concourse.mybir as mybir
import concourse.tile as tile
from concourse import bass_utils

# ---- problem constants (hardcoded; kernel.py must be self-contained) ----
GRID = (64, 64, 64)
NX, NY, NZ = GRID
N = NX * NY * NZ
RADIUS = 2
NCORES = 8
YS = NY // NCORES  # y-planes per core = 8

TAU = 20.0
REST_V = -65.0
EXC_THR = -50.0
INH_THR = -70.0
RESET_V = -65.0
ETA_LTP, ETA_LTD, WDECAY = 0.01, 0.005, 1e-05

# fp32-exact scalars matching the jax reference
DECAY = float(np.exp(np.float32(-1.0 / np.float32(TAU))).astype(np.float32))
ONE_MINUS_DECAY = float(np.float32(1.0) - np.float32(DECAY))
MIDPOINT = (EXC_THR + INH_THR) / 2.0  # -60.0

# offsets sorted by descending linear shift (matches reference edge order:
# for a destination j, contributions are summed over ascending source i)
OFFSETS = sorted(
    [
        (dx, dy, dz)
        for dx in range(-RADIUS, RADIUS + 1)
        for dy in range(-RADIUS, RADIUS + 1)
        for dz in range(-RADIUS, RADIUS + 1)
        if 0 < abs(dx) + abs(dy) + abs(dz) <= RADIUS
    ],
    key=lambda d: -(d[0] * NY * NZ + d[1] * NZ + d[2]),
)
NOFF = len(OFFSETS)  # 24
XSHIFTS = (1, -1, 2, -2)

# field geometry: partition p = x (64), free = ys*68 + zs,
# ys = y_loc + 2 in [0,12), zs = z + 2 in [0,68)
FYS = 12           # field y rows (8 own + 2 halo each side)
FZS = 68           # field z cols (64 + 2 pad each side)
FFREE = FYS * FZS  # 816
CHUNK = YS * NZ    # 512 elems per partition for compact tiles
F32 = mybir.dt.float32

_CACHE = {}


def _build_graph(nsteps):
    nc = bacc.Bacc(
        "TRN2",
        target_bir_lowering=False,
        debug=False,
        enable_asserts=True,
        num_devices=NCORES,
    )
    w0_d = nc.dram_tensor("w0", [NX, NOFF * CHUNK], F32, kind="ExternalInput").ap()
    xin_d = nc.dram_tensor("xin", [nsteps, NX, CHUNK], F32, kind="ExternalInput").ap()
    msk_d = nc.dram_tensor("msk", [NX, 2], F32, kind="ExternalInput").ap()
    spk_d = nc.dram_tensor("spk", [nsteps, NX, CHUNK], F32, kind="ExternalOutput").ap()

    AT = mybir.ActivationFunctionType
    ALU = mybir.AluOpType

    with tile.TileContext(nc) as tc, tc.tile_pool(
        name="state", bufs=1
    ) as st, tc.tile_pool(name="dram", bufs=1, space="DRAM") as dr:
        # persistent state tiles (distinct names -> distinct slots)
        W = st.tile([NX, NOFF * CHUNK], F32, name="W")
        P = st.tile([NX, NOFF * CHUNK], F32, name="P")
        FA = st.tile([NX, FFREE], F32, name="FA")
        FB = st.tile([NX, FFREE], F32, name="FB")
        QF = st.tile([NX, FFREE], F32, name="QF")
        SO = {dx: st.tile([NX, FFREE], F32, name=f"SOx{dx+2}") for dx in XSHIFTS}
        SQ = {dx: st.tile([NX, FFREE], F32, name=f"SQx{dx+2}") for dx in XSHIFTS}
        V = st.tile([NX, CHUNK], F32, name="V")
        SYN = st.tile([NX, CHUNK], F32, name="SYN")
        S = st.tile([NX, CHUNK], F32, name="S")
        II = st.tile([NX, CHUNK], F32, name="II")
        G = st.tile([NX, CHUNK], F32, name="G")
        NN = st.tile([NX, CHUNK], F32, name="NN")
        T1 = st.tile([NX, CHUNK], F32, name="T1")
        XIP = st.tile([NX, CHUNK], F32, name="XIP")
        RST = st.tile([NX, CHUNK], F32, name="RST")
        MSK = st.tile([NX, 2], F32, name="MSK")
        SND = st.tile([NX, 4 * NZ], F32, name="SND")
        B30 = st.tile([NX, 1], F32, name="B30")  # sigmoid bias const
        SI = st.tile([NX, CHUNK], mybir.dt.uint8, name="SI")  # int spike mask

        def f3(t):  # [NX, FFREE] -> [NX, FYS, FZS]
            return t.rearrange("p (y z) -> p y z", z=FZS)

        def c3(t):  # [NX, CHUNK] -> [NX, YS, NZ]
            return t.rearrange("p (y z) -> p y z", z=NZ)

        def w3(t, k):  # [NX, NOFF*CHUNK] slot k -> [NX, YS, NZ]
            return t.rearrange("p (k y z) -> p k y z", k=NOFF, z=NZ)[:, k]

        def fint(t):  # own interior of a field tile -> [NX, YS, NZ]
            return f3(t)[:, 2 : 2 + YS, 2 : 2 + NZ]

        def fshift(base, xs, d):
            # source view reading position j - d, with x-shift via copies
            dx, dy, dz = d
            t = base if dx == 0 else xs[dx]
            return f3(t)[:, 2 - dy : 2 - dy + YS, 2 - dz : 2 - dz + NZ]

        def xshift_dma(dst, src, dx):
            # dst[p] = src[p - dx] for the valid range (edge rows stay 0)
            a, b = max(0, dx), NX + min(0, dx)
            nc.sync.dma_start(dst[a:b, :], src[a - dx : b - dx, :])

        # ---- init ----
        nc.vector.memset(FA[:], 0.0)
        nc.vector.memset(FB[:], 0.0)
        nc.vector.memset(QF[:], 0.0)
        for dx in XSHIFTS:
            nc.vector.memset(SO[dx][:], 0.0)
            nc.vector.memset(SQ[dx][:], 0.0)
        nc.vector.memset(V[:], REST_V)
        nc.vector.memset(B30[:], -0.5 * MIDPOINT)
        nc.vector.memset(RST[:], RESET_V)
        nc.sync.dma_start(W[:], w0_d[:])
        nc.sync.dma_start(MSK[:], msk_d[:])

        pid = nc.sync.partition_id()
        offL = nc.sync.snap((pid + NCORES - 1) % NCORES, min_val=0, max_val=NCORES - 1)
        offR = nc.sync.snap((pid + 1) % NCORES, min_val=0, max_val=NCORES - 1)

        fields = [FA, FB]
        for t in range(nsteps):
            FPREV = fields[t % 2]       # holds out_{t-1} (with halos)
            FOUT = fields[(t + 1) % 2]  # will hold out_t

            # external input, prescaled by (1-decay)
            nc.sync.dma_start(XIP[:], xin_d[t])
            nc.scalar.mul(XIP[:], XIP[:], ONE_MINUS_DECAY)

            # ---- syn: 24 shifted products then pairwise tree sum ----
            for k, d in enumerate(OFFSETS):
                nc.vector.tensor_tensor(
                    w3(P, k), w3(W, k), fshift(FPREV, SO, d), ALU.mult
                )
            # tree-reduce the 24 slots (in place over P)
            Pf = P.rearrange("p (k f) -> p k f", k=NOFF)
            nslots = NOFF
            while nslots > 1:
                half = nslots // 2
                nc.vector.tensor_tensor(
                    Pf[:, 0:half],
                    Pf[:, 0:half],
                    Pf[:, half : 2 * half],
                    ALU.add,
                )
                if nslots % 2:
                    nc.vector.tensor_tensor(
                        Pf[:, 0], Pf[:, 0], Pf[:, nslots - 1], ALU.add
                    )
                nslots = half
            # SYN = (P0 * (1-decay)) + XIP ; then v = v*decay + SYN
            nc.vector.scalar_tensor_tensor(
                SYN[:], Pf[:, 0], ONE_MINUS_DECAY, XIP[:], ALU.mult, ALU.add
            )
            nc.vector.scalar_tensor_tensor(
                V[:], V[:], DECAY, SYN[:], ALU.mult, ALU.add
            )

            # ---- neuron update ----
            nc.gpsimd.tensor_single_scalar(S[:], V[:], EXC_THR, ALU.is_ge)
            nc.gpsimd.tensor_single_scalar(SI[:], V[:], EXC_THR, ALU.is_ge)
            nc.gpsimd.tensor_single_scalar(II[:], V[:], INH_THR, ALU.is_le)
            nc.scalar.activation(G[:], V[:], AT.Sigmoid, bias=B30[:, 0:1], scale=0.5)
            nc.gpsimd.tensor_tensor(NN[:], S[:], II[:], ALU.add)
            nc.vector.scalar_tensor_tensor(
                T1[:], NN[:], 1.0, G[:], ALU.subtract, ALU.mult
            )
            # out = s - (n-1)*g, written straight into the FOUT interior
            nc.vector.tensor_tensor(fint(FOUT), c3(S), c3(T1), ALU.subtract)
            # v reset where spiked
            nc.vector.copy_predicated(V[:], SI[:], RST[:])
            # spike train output for this step
            nc.sync.dma_start(spk_d[t], S[:])

            # ---- halo exchange of out_t ----
            snd3 = SND.rearrange("p (y z) -> p y z", z=NZ)
            # top 2 own rows (y_loc 6,7) masked by col-0, bottom 2 (y_loc 0,1) by col-1
            nc.scalar.activation(
                snd3[:, 0:2], f3(FOUT)[:, 8:10, 2 : 2 + NZ],
                AT.Copy, bias=0.0, scale=MSK[:, 0:1],
            )
            nc.scalar.activation(
                snd3[:, 2:4], f3(FOUT)[:, 2:4, 2 : 2 + NZ],
                AT.Copy, bias=0.0, scale=MSK[:, 1:2],
            )
            agin = dr.tile([NX, 4 * NZ], F32, name=f"agin{t}")
            agout = dr.tile(
                [NCORES * NX, 4 * NZ], F32, addr_space="Shared", name=f"agout{t}"
            )
            nc.sync.dma_start(agin[:], SND[:])
            nc.gpsimd.collective_compute(
                "AllGather",
                ALU.bypass,
                replica_groups=[list(range(NCORES))],
                ins=[agin.opt()],
                outs=[agout.opt()],
            )
            agf = agout.rearrange("p (y z) -> p y z", z=NZ)
            # left neighbor's top strip -> my bottom halo rows (ys 0,1)
            nc.sync.dma_start(
                f3(FOUT)[:, 0:2, 2 : 2 + NZ],
                agf[bass.ds(offL * NX, NX), 0:2],
            )
            # right neighbor's bottom strip -> my top halo rows (ys 10,11)
            nc.sync.dma_start(
                f3(FOUT)[:, 10:12, 2 : 2 + NZ],
                agf[bass.ds(offR * NX, NX), 2:4],
            )

            # x-shifted copies of out_t (serve step t+1 syn and step t STDP)
            for dx in XSHIFTS:
                xshift_dma(SO[dx], FOUT, dx)

            # ---- STDP (skipped at t=0, matching the reference) ----
            if t > 0:
                # q = 0.015*out_t - 0.005 over the full halo'd field
                nc.scalar.activation(
                    QF[:], FOUT[:], AT.Copy, bias=-ETA_LTD, scale=ETA_LTP + ETA_LTD
                )
                for dx in XSHIFTS:
                    xshift_dma(SQ[dx], QF, dx)
                for k, d in enumerate(OFFSETS):
                    nc.vector.tensor_tensor(
                        w3(P, k), fint(FPREV), fshift(QF, SQ, d), ALU.mult
                    )
                nc.vector.scalar_tensor_tensor(
                    W[:], W[:], 1.0 - WDECAY, P[:], ALU.mult, ALU.add
                )
                nc.vector.tensor_scalar(W[:], W[:], 1.0, 0.0, ALU.min, ALU.max)

    nc.compile()
    return nc


def _shard_inputs(external_input, edge_values, edge_rows, edge_cols, nsteps):
    """Build per-core input maps (host-side sharding)."""
    ext = np.ascontiguousarray(np.asarray(external_input, dtype=np.float32))[:nsteps]
    vals = np.asarray(edge_values, dtype=np.float32)
    rows = np.asarray(edge_rows, dtype=np.int64)
    cols = np.asarray(edge_cols, dtype=np.int64)

    # dense weights keyed by destination: Wd[k, j] = w(edge j-d_k -> j)
    dlin = cols - rows
    offs_lin = np.array([d[0] * NY * NZ + d[1] * NZ + d[2] for d in OFFSETS])
    assert set(int(v) for v in np.unique(dlin)).issubset(
        set(int(v) for v in offs_lin)
    )
    k_of = np.zeros(int(offs_lin.max()) - int(offs_lin.min()) + 1, dtype=np.int64)
    for i, v in enumerate(offs_lin):
        k_of[int(v) - int(offs_lin.min())] = i
    ke = k_of[dlin - int(offs_lin.min())]
    Wd = np.zeros((NOFF, N), dtype=np.float32)
    Wd[ke, cols] = vals

    Wd = Wd.reshape(NOFF, NX, NY, NZ)
    ext = ext.reshape(nsteps, NX, NY, NZ)

    in_maps = []
    for c in range(NCORES):
        ylo = c * YS
        wc = np.ascontiguousarray(
            Wd[:, :, ylo : ylo + YS, :].transpose(1, 0, 2, 3)
        ).reshape(NX, NOFF * CHUNK)
        xc = np.ascontiguousarray(ext[:, :, ylo : ylo + YS, :]).reshape(
            nsteps, NX, CHUNK
        )
        msk = np.zeros((NX, 2), dtype=np.float32)
        msk[:, 0] = 0.0 if c == NCORES - 1 else 1.0  # top strip valid?
        msk[:, 1] = 0.0 if c == 0 else 1.0           # bottom strip valid?
        in_maps.append({"w0": wc, "xin": xc, "msk": msk})
    return in_maps


def kernel(external_input, edge_values, edge_rows, edge_cols, num_steps):
    nsteps = int(num_steps)
    if nsteps not in _CACHE:
        _CACHE[nsteps] = _build_graph(nsteps)
    nc = _CACHE[nsteps]

    in_maps = _shard_inputs(external_input, edge_values, edge_rows, edge_cols, nsteps)
    res = bass_utils.run_bass_kernel_spmd(
        nc,
        in_maps,
        core_ids=list(range(NCORES)),
        trace=bool(int(os.environ.get("BRAIN_TRACE", "0"))),
    )

    out = np.empty((nsteps, NX, NY, NZ), dtype=np.float32)
    for c in range(NCORES):
        ylo = c * YS
        out[:, :, ylo : ylo + YS, :] = res.results[c]["spk"].reshape(
            nsteps, NX, YS, NZ
        )
    kernel.last_results = res
    return out.reshape(nsteps, N)



# revision 2
# speedup vs baseline: 4.1588x; 4.1588x over previous
"""Trainium2 Bass kernel for Brain3DQTUNNetwork (gnn_message_passing), v2.

Structure (per core, y-slab of 8 planes, 128 partitions p = h*64 + x with
h = y-half):

- Weights are stored SOURCE-x-aligned per offset slot k: W'_k[i] = W_k[i+dx],
  so the per-step SpMV reads the prev field with only (dy,dz) free-dim
  shifts (no shifted-field copies).  The per-dx partial sums are realigned
  to destinations by 4 tiny PE matmuls with banded 0/1 shift matrices,
  accumulated in PSUM.
- Per-slot products/tree/weights run in fp16 (DVE 2x_1p / 4x_2p modes);
  the V/neuron path stays fp32.  Weight decay is folded into a global
  scale c_t = (1-wd)^t (W~ = W/c_t), so the STDP update is a plain fp16
  tensor_tensor add + a 2-op tensor_scalar clip.
- Per step the only cross-core traffic is an AllGather of the 2-row
  y-boundary strips; STDP work on halo-free slots overlaps it.
- external_input is preloaded to SBUF (prescaled by (1-decay)) for all
  steps; spikes DMA out per step.
"""

import os
import sys

sys.path.insert(0, "/opt/trn_rl_repo")

import numpy as np

import concourse.bass as bass
import concourse.bacc as bacc
import concourse.mybir as mybir
import concourse.tile as tile
from concourse import bass_utils
from bass_rust import AP as RawAP

# ---- problem constants (hardcoded; kernel.py must be self-contained) ----
GRID = (64, 64, 64)
NX, NY, NZ = GRID
N = NX * NY * NZ
RADIUS = 2
NCORES = 8
YS = NY // NCORES  # 8 y-planes per core
YH = YS // 2       # 4 rows per partition half

TAU = 20.0
REST_V = -65.0
EXC_THR = -50.0
INH_THR = -70.0
RESET_V = -65.0
ETA_LTP, ETA_LTD, WDECAY = 0.01, 0.005, 1e-05

DECAY = float(np.exp(np.float32(-1.0 / np.float32(TAU))).astype(np.float32))
ONE_MINUS_DECAY = float(np.float32(1.0) - np.float32(DECAY))
MIDPOINT = (EXC_THR + INH_THR) / 2.0  # -60.0

# slot table: (dx, dy, dz) in the fixed kernel order.  Grouping invariants:
#  * slots of one (dx, dy, dz-run) are k-consecutive (batched mult groups)
#  * each dx block is k-consecutive enough for the partial-sum tree
#  * halo-free slots (dy == 0) sit in ranges [0:3),[5:8),[10:14),[22:24)
SLOTS = [
    (1, 0, 1), (1, 0, 0), (1, 0, -1),          # k0-2
    (1, 1, 0), (1, -1, 0),                     # k3, k4
    (-1, 0, 1), (-1, 0, 0), (-1, 0, -1),       # k5-7
    (-1, 1, 0), (-1, -1, 0),                   # k8, k9
    (0, 0, 2), (0, 0, 1),                      # k10-11
    (0, 0, -1), (0, 0, -2),                    # k12-13
    (0, 1, 1), (0, 1, 0), (0, 1, -1),          # k14-16
    (0, -1, 1), (0, -1, 0), (0, -1, -1),       # k17-19
    (0, 2, 0), (0, -2, 0),                     # k20, k21
    (2, 0, 0), (-2, 0, 0),                     # k22, k23
]
NOFF = len(SLOTS)  # 24

# mult groups: (k0, L, dy, dz_start) — slots k0..k0+L-1 share (dx, dy) and
# have dz = dz_start, dz_start-1, ...; field z-offset 2-dz has k-stride +1.
GROUPS_HF = [
    (0, 3, 0, 1), (5, 3, 0, 1),
    (10, 2, 0, 2), (12, 2, 0, -1),
    (22, 1, 0, 0), (23, 1, 0, 0),
]
GROUPS_H = [
    (3, 1, 1, 0), (4, 1, -1, 0),
    (8, 1, 1, 0), (9, 1, -1, 0),
    (14, 3, 1, 1), (17, 3, -1, 1),
    (20, 1, 2, 0), (21, 1, -2, 0),
]
GROUPS = GROUPS_HF + GROUPS_H

# prev-side source for each group: dx value (0 -> FPREV interior, else PVX)
GROUP_DX = {0: 1, 5: -1, 10: 0, 12: 0, 22: 2, 23: -2,
            3: 1, 4: 1, 8: -1, 9: -1, 14: 0, 17: 0, 20: 0, 21: 0}
PVX_SLICE = {1: 0, -1: 1, 2: 2, -2: 3}

FZ = NZ + 4        # 68 field z cols
FR = 2 * YH        # 8 field rows
FFREE = FR * FZ    # 544
CH = YH * NZ       # 256 own cells per partition

F32 = mybir.dt.float32
U8 = mybir.dt.uint8

# product/weight dtype: fp16 enables DVE 2x_1p/4x_2p fast modes
USE_F16 = bool(int(os.environ.get("BRAIN_F16", "1")))
DT = mybir.dt.float16 if USE_F16 else mybir.dt.float32
NPDT = np.float16 if USE_F16 else np.float32

_CACHE = {}


def _overlap_ap(view, kstride, ksize):
    """Insert a k dim (kstride in free elems) after the partition dim."""
    ap = [list(d) for d in view.ap]
    ap.insert(1, [kstride, ksize])
    return RawAP(tensor=view.tensor, offset=view.offset, ap=ap)


def _blockpair_ap(view, blkstride, nblk):
    """Insert a leading block dim covering two disjoint equal k-ranges."""
    ap = [list(d) for d in view.ap]
    ap.insert(1, [blkstride, nblk])
    return RawAP(tensor=view.tensor, offset=view.offset, ap=ap)


def _build_graph(nsteps):
    nc = bacc.Bacc(
        "TRN2",
        target_bir_lowering=False,
        debug=False,
        enable_asserts=True,
        num_devices=NCORES,
    )
    P128 = 2 * NX
    w0_d = nc.dram_tensor("w0", [P128, NOFF * CH], DT, kind="ExternalInput").ap()
    xin_d = nc.dram_tensor("xin", [P128, nsteps * CH], F32, kind="ExternalInput").ap()
    msk_d = nc.dram_tensor("msk", [P128, 1], F32, kind="ExternalInput").ap()
    sm_d = nc.dram_tensor("sm", [P128, 4 * P128], DT, kind="ExternalInput").ap()
    spk_d = nc.dram_tensor("spk", [nsteps, P128, CH], F32, kind="ExternalOutput").ap()

    AT = mybir.ActivationFunctionType
    ALU = mybir.AluOpType

    # per-step W~ scale bookkeeping (c_t = (1-wd)^t, W~ = W / c_t)
    c = [float(np.float64(1.0 - WDECAY) ** t) for t in range(nsteps)]

    with tile.TileContext(nc) as tc, tc.tile_pool(
        name="state", bufs=1
    ) as st, tc.tile_pool(name="psum", bufs=1, space="PSUM") as ps, tc.tile_pool(
        name="dram", bufs=1, space="DRAM"
    ) as dr:
        W = st.tile([P128, NOFF * CH], DT, name="W")
        P = st.tile([P128, NOFF * CH], DT, name="P")
        FA = st.tile([P128, FFREE], DT, name="FA")
        FB = st.tile([P128, FFREE], DT, name="FB")
        QF = st.tile([P128, FFREE], DT, name="QF")
        PVX = st.tile([P128, 4 * CH], DT, name="PVX")
        SM = st.tile([P128, 4 * P128], DT, name="SM")
        XINP = st.tile([P128, nsteps * CH], F32, name="XINP")
        V = st.tile([P128, CH], F32, name="V")
        SYN = st.tile([P128, CH], F32, name="SYN")
        SS = [st.tile([P128, CH], F32, name=f"S{i}") for i in range(2)]
        SI = st.tile([P128, CH], U8, name="SI")
        II = st.tile([P128, CH], F32, name="II")
        G = st.tile([P128, CH], F32, name="G")
        E = st.tile([P128, CH], F32, name="E")
        RST = st.tile([P128, CH], F32, name="RST")
        B30 = st.tile([P128, 1], F32, name="B30")
        MSK = st.tile([P128, 1], F32, name="MSK")
        SND = st.tile([P128, 2 * NZ], DT, name="SND")
        TDP = ps.tile([P128, CH], F32, name="TDP")

        def f3(t):  # field [p, FR, FZ]
            return t.rearrange("p (r z) -> p r z", z=FZ)

        def c3(t):  # chunk [p, YH, NZ]
            return t.rearrange("p (y z) -> p y z", z=NZ)

        def w4(t, k0, L):  # W/P slots [p, L, YH, NZ]
            return t.rearrange("p (k y z) -> p k y z", k=NOFF, z=NZ)[:, k0 : k0 + L]

        def wf(t, k0, k1):  # W/P slot range [p, (k1-k0)*CH] flat
            return t.rearrange("p (k f) -> p k f", k=NOFF)[:, k0:k1]

        def fint(t):  # own interior [p, YH, NZ]
            return f3(t)[:, 2 : 2 + YH, 2 : 2 + NZ]

        def grp_field(t, L, dy, dz0):
            # [p, L, YH, NZ] reading rows 2-dy.., z cols (2-dz0).., k-stride +1
            base = f3(t)[:, 2 - dy : 2 - dy + YH, 2 - dz0 : 2 - dz0 + NZ]
            if L == 1:
                return base.unsqueeze(1)
            return _overlap_ap(base, 1, L)

        def prev_bcast(t, L, dx):
            # prev-side operand for a group: [p, L, YH, NZ] broadcast over k
            if dx == 0:
                v = fint(t)
            else:
                v = c3(PVX.rearrange("p (s f) -> p s f", s=4)[:, PVX_SLICE[dx]])
            return v.unsqueeze(1).to_broadcast([P128, L, YH, NZ])

        def xshift_dma(eng, dst, src, dx):
            # dst[p] = src[p - dx] within each 64-partition h-block
            for h in (0, 1):
                a, b = h * NX + max(0, dx), h * NX + NX + min(0, dx)
                eng.dma_start(dst[a:b], src[a - dx : b - dx])

        # ---- init ----
        nc.vector.memset(FA[:], 0.0)
        nc.vector.memset(FB[:], 0.0)
        nc.vector.memset(PVX[:], 0.0)
        nc.vector.memset(V[:], REST_V)
        nc.vector.memset(RST[:], RESET_V)
        nc.vector.memset(B30[:], -0.5 * MIDPOINT)
        nc.sync.dma_start(W[:], w0_d[:])
        nc.sync.dma_start(SM[:], sm_d[:])
        nc.sync.dma_start(MSK[:], msk_d[:])
        nc.sync.dma_start(XINP[:], xin_d[:])

        pid = nc.sync.partition_id()
        offL = nc.sync.snap((pid + NCORES - 1) % NCORES, min_val=0, max_val=NCORES - 1)
        offR = nc.sync.snap((pid + 1) % NCORES, min_val=0, max_val=NCORES - 1)

        XV = XINP.rearrange("p (t f) -> p t f", t=nsteps)
        fields = [FA, FB]
        for t in range(nsteps):
            FPREV = fields[t % 2]
            FOUT = fields[(t + 1) % 2]
            last = t == nsteps - 1
            S = SS[t % 2]

            # prefetch x-shifted prev chunks for this step's STDP
            if 0 < t and not last:
                pvs = PVX.rearrange("p (s f) -> p s f", s=4)
                for dxv, sl in PVX_SLICE.items():
                    xshift_dma(nc.scalar, pvs[:, sl], fint(FPREV), -dxv)

            # ---- syn: per-slot products, per-dx trees, PE realign ----
            if t > 0:
                for (k0, L, dy, dz0) in GROUPS:
                    nc.vector.tensor_tensor(
                        w4(P, k0, L), w4(W, k0, L), grp_field(FPREV, L, dy, dz0),
                        ALU.mult,
                    )
                Pk = P.rearrange("p (k f) -> p k f", k=NOFF)

                def padd(d0, d1, s0, s1):
                    nc.vector.tensor_tensor(
                        Pk[:, d0:d1], Pk[:, d0:d1], Pk[:, s0:s1], ALU.add
                    )

                padd(0, 2, 2, 4)    # dx=+1 block
                padd(0, 1, 1, 2)
                padd(0, 1, 4, 5)
                padd(5, 7, 7, 9)    # dx=-1 block
                padd(5, 6, 6, 7)
                padd(5, 6, 9, 10)
                padd(10, 16, 16, 22)  # dx=0 block
                padd(10, 13, 13, 16)
                padd(10, 11, 11, 12)
                padd(10, 11, 12, 13)
                smv = SM.rearrange("p (s m) -> p s m", s=4)
                for i, (slot, sl) in enumerate(
                    ((0, 0), (5, 1), (22, 2), (23, 3))
                ):
                    nc.tensor.matmul(
                        TDP[:], smv[:, sl], Pk[:, slot],
                        start=(i == 0), stop=(i == 3),
                    )
                nc.vector.tensor_tensor(SYN[:], Pk[:, 10], TDP[:], ALU.add)
                s_t = float(np.float32(ONE_MINUS_DECAY * c[t - 1]))
                nc.vector.scalar_tensor_tensor(
                    SYN[:], SYN[:], s_t, XV[:, t], ALU.mult, ALU.add
                )
                nc.vector.scalar_tensor_tensor(
                    V[:], V[:], DECAY, SYN[:], ALU.mult, ALU.add
                )
            else:
                nc.vector.scalar_tensor_tensor(
                    V[:], V[:], DECAY, XV[:, t], ALU.mult, ALU.add
                )

            # ---- neuron update ----
            nc.vector.tensor_scalar(S[:], V[:], EXC_THR, None, ALU.is_ge)
            nc.sync.dma_start(spk_d[t], S[:])
            if not last:
                nc.vector.tensor_scalar(SI[:], V[:], EXC_THR, None, ALU.is_ge)
                nc.vector.tensor_scalar(II[:], V[:], INH_THR, None, ALU.is_le)
                nc.scalar.activation(G[:], V[:], AT.Sigmoid, bias=B30[:, 0:1], scale=0.5)
                nc.vector.tensor_tensor(E[:], S[:], II[:], ALU.subtract)
                nc.vector.tensor_tensor(E[:], G[:], E[:], ALU.add)
                # out = clip01(sigmoid + spike - inhibited), written to FOUT
                nc.vector.tensor_scalar(fint(FOUT), c3(E), 1.0, 0.0, ALU.min, ALU.max)
                nc.vector.copy_predicated(V[:], SI[:], RST[:])

                # ---- boundary strips -> AllGather (overlapped below) ----
                snd3 = SND.rearrange("p (r z) -> p r z", z=NZ)
                nc.scalar.activation(
                    snd3[0:NX], f3(FOUT)[0:NX, 2:4, 2 : 2 + NZ],
                    AT.Copy, bias=0.0, scale=MSK[0:NX, 0:1],
                )
                nc.scalar.activation(
                    snd3[NX:P128], f3(FOUT)[NX:P128, 4:6, 2 : 2 + NZ],
                    AT.Copy, bias=0.0, scale=MSK[NX:P128, 0:1],
                )
                agin = dr.tile([P128, 2 * NZ], DT, name=f"agin{t}")
                agout = dr.tile(
                    [NCORES * P128, 2 * NZ], DT, addr_space="Shared", name=f"agout{t}"
                )
                nc.sync.dma_start(agin[:], SND[:])
                nc.gpsimd.collective_compute(
                    "AllGather",
                    ALU.bypass,
                    replica_groups=[list(range(NCORES))],
                    ins=[agin.opt()],
                    outs=[agout.opt()],
                )
                # intra-core y halos (own h-half exchange)
                nc.sync.dma_start(f3(FOUT)[0:NX, 6:8, 2 : 2 + NZ],
                                  f3(FOUT)[NX:P128, 2:4, 2 : 2 + NZ])
                nc.sync.dma_start(f3(FOUT)[NX:P128, 0:2, 2 : 2 + NZ],
                                  f3(FOUT)[0:NX, 4:6, 2 : 2 + NZ])

            # ---- STDP halo-free part (overlaps the collective) ----
            do_stdp = 0 < t and not last
            if do_stdp:
                a_t = float(np.float32((ETA_LTP + ETA_LTD) / c[t]))
                b_t = float(np.float32(-ETA_LTD / c[t]))
                hi_t = float(np.float32(1.0 / c[t]))
                qf3 = f3(QF)
                fo3 = f3(FOUT)
                nc.scalar.activation(
                    qf3[:, 2:6], fo3[:, 2:6], AT.Copy, bias=b_t, scale=a_t
                )
                for (k0, L, dy, dz0) in GROUPS_HF:
                    nc.vector.tensor_tensor(
                        w4(P, k0, L), prev_bcast(FPREV, L, GROUP_DX[k0]),
                        grp_field(QF, L, dy, dz0), ALU.mult,
                    )
                # W~ += P2 ; clip [0, 1/c_t]  (halo-free slot ranges)
                for rng in (((0, 3), (5, 8)), ((10, 14), None), ((22, 24), None)):
                    (r0, r1), pair = rng
                    wv, pv = wf(W, r0, r1), wf(P, r0, r1)
                    if pair is not None:
                        blk = (pair[0] - r0) * CH
                        wv = _blockpair_ap(wf(W, r0, r1), blk, 2)
                        pv = _blockpair_ap(wf(P, r0, r1), blk, 2)
                    nc.vector.tensor_tensor(wv, wv, pv, ALU.add)
                    nc.vector.tensor_scalar(wv, wv, hi_t, 0.0, ALU.min, ALU.max)

            # ---- halo in from neighbors ----
            if not last:
                agf = agout.rearrange("p (r z) -> p r z", z=NZ)
                nc.sync.dma_start(
                    f3(FOUT)[0:NX, 0:2, 2 : 2 + NZ],
                    agf[bass.ds(offL * P128 + NX, NX)],
                )
                nc.sync.dma_start(
                    f3(FOUT)[NX:P128, 6:8, 2 : 2 + NZ],
                    agf[bass.ds(offR * P128, NX)],
                )

            # ---- STDP halo part ----
            if do_stdp:
                nc.scalar.activation(
                    qf3[:, 0:2], fo3[:, 0:2], AT.Copy, bias=b_t, scale=a_t
                )
                nc.scalar.activation(
                    qf3[:, 6:8], fo3[:, 6:8], AT.Copy, bias=b_t, scale=a_t
                )
                for (k0, L, dy, dz0) in GROUPS_H:
                    nc.vector.tensor_tensor(
                        w4(P, k0, L), prev_bcast(FPREV, L, GROUP_DX[k0]),
                        grp_field(QF, L, dy, dz0), ALU.mult,
                    )
                for rng in (((3, 5), (8, 10)), ((14, 22), None)):
                    (r0, r1), pair = rng
                    wv, pv = wf(W, r0, r1), wf(P, r0, r1)
                    if pair is not None:
                        blk = (pair[0] - r0) * CH
                        wv = _blockpair_ap(wf(W, r0, r1), blk, 2)
                        pv = _blockpair_ap(wf(P, r0, r1), blk, 2)
                    nc.vector.tensor_tensor(wv, wv, pv, ALU.add)
                    nc.vector.tensor_scalar(wv, wv, hi_t, 0.0, ALU.min, ALU.max)

    nc.compile()
    return nc


def _shard_inputs(external_input, edge_values, edge_rows, edge_cols, nsteps):
    ext = np.ascontiguousarray(np.asarray(external_input, dtype=np.float32))[:nsteps]
    vals = np.asarray(edge_values, dtype=np.float32)
    rows = np.asarray(edge_rows, dtype=np.int64)
    cols = np.asarray(edge_cols, dtype=np.int64)

    # dest-keyed dense weights in kernel slot order
    dlin = cols - rows
    offs_lin = np.array([d[0] * NY * NZ + d[1] * NZ + d[2] for d in SLOTS])
    k_of = {int(v): i for i, v in enumerate(offs_lin)}
    ke = np.array([k_of[int(v)] for v in dlin], dtype=np.int64)
    Wd = np.zeros((NOFF, N), dtype=np.float32)
    Wd[ke, cols] = vals
    Wd = Wd.reshape(NOFF, NX, NY, NZ)

    # source-x-aligned: W'_k[x] = Wd_k[x+dx]
    Wsrc = np.zeros_like(Wd)
    for k, (dx, _, _) in enumerate(SLOTS):
        if dx >= 0:
            Wsrc[k, : NX - dx] = Wd[k, dx:]
        else:
            Wsrc[k, -dx:] = Wd[k, : NX + dx]

    ext4 = ext.reshape(nsteps, NX, NY, NZ) * np.float32(ONE_MINUS_DECAY)

    # PE shift matrices: SM_s[p, m] = 1 iff m = p + dx (same h block)
    sm = np.zeros((2 * NX, 4, 2 * NX), dtype=np.float32)
    for s, dxv in enumerate((1, -1, 2, -2)):
        for h in (0, 1):
            for xs in range(NX):
                xm = xs + dxv
                if 0 <= xm < NX:
                    sm[h * NX + xs, s, h * NX + xm] = 1.0
    sm = sm.reshape(2 * NX, 4 * 2 * NX).astype(NPDT)

    in_maps = []
    for cidx in range(NCORES):
        ylo = cidx * YS
        sub = Wsrc[:, :, ylo : ylo + YS, :]          # [24, 64, 8, 64]
        tr = sub.transpose(1, 0, 2, 3)               # [64, 24, 8, 64]
        wc = np.concatenate(
            [tr[:, :, :YH, :].reshape(NX, NOFF * CH),
             tr[:, :, YH:, :].reshape(NX, NOFF * CH)], axis=0
        ).astype(NPDT)
        esub = ext4[:, :, ylo : ylo + YS, :].transpose(1, 0, 2, 3)  # [64,T,8,64]
        xc = np.concatenate(
            [esub[:, :, :YH, :].reshape(NX, nsteps * CH),
             esub[:, :, YH:, :].reshape(NX, nsteps * CH)], axis=0
        ).astype(np.float32)
        msk = np.ones((2 * NX, 1), dtype=np.float32)
        if cidx == 0:
            msk[:NX] = 0.0
        if cidx == NCORES - 1:
            msk[NX:] = 0.0
        in_maps.append(
            {"w0": np.ascontiguousarray(wc), "xin": np.ascontiguousarray(xc),
             "msk": msk, "sm": sm}
        )
    return in_maps


def kernel(external_input, edge_values, edge_rows, edge_cols, num_steps):
    nsteps = int(num_steps)
    if nsteps not in _CACHE:
        _CACHE[nsteps] = _build_graph(nsteps)
    nc = _CACHE[nsteps]

    in_maps = _shard_inputs(external_input, edge_values, edge_rows, edge_cols, nsteps)
    res = bass_utils.run_bass_kernel_spmd(
        nc,
        in_maps,
        core_ids=list(range(NCORES)),
        trace=bool(int(os.environ.get("BRAIN_TRACE", "0"))),
    )

    out = np.empty((nsteps, NX, NY, NZ), dtype=np.float32)
    for cidx in range(NCORES):
        ylo = cidx * YS
        spk = res.results[cidx]["spk"].reshape(nsteps, 2, NX, YH, NZ)
        out[:, :, ylo : ylo + YH, :] = spk[:, 0].transpose(0, 1, 2, 3)
        out[:, :, ylo + YH : ylo + YS, :] = spk[:, 1]
    kernel.last_results = res
    return out.reshape(nsteps, N)


# revision 3
# speedup vs baseline: 4.5164x; 1.0860x over previous
"""Trainium2 Bass kernel for Brain3DQTUNNetwork (gnn_message_passing), v3.

Per core: y-slab of 8 planes, 128 partitions p = h*64 + x (h = y-half).
Weights stored SOURCE-x-aligned per offset slot, so the SpMV reads the prev
field with only (dy,dz) free-dim shifts; per-slot products are realigned to
destinations and k-reduced by 24 tiny PE matmuls (banded 0/1 shift matrices
/ identity) accumulating in PSUM.  Products/weights run fp16 (DVE 2x_1p /
4x_2p); V/neuron path fp32.  Weight decay is folded into a global scale
c_t = (1-wd)^t so the STDP update is a plain fp16 add + 2-op clip.
Cross-core traffic: one AllGather of 2-row boundary strips per step,
overlapped with halo-free STDP and the NEXT step's halo-free syn products
(software pipelining).
"""

import os
import sys

sys.path.insert(0, "/opt/trn_rl_repo")

import numpy as np

import concourse.bass as bass
import concourse.bacc as bacc
import concourse.mybir as mybir
import concourse.tile as tile
from concourse import bass_utils
from bass_rust import AP as RawAP

# ---- problem constants (hardcoded; kernel.py must be self-contained) ----
GRID = (64, 64, 64)
NX, NY, NZ = GRID
N = NX * NY * NZ
RADIUS = 2
NCORES = 8
YS = NY // NCORES  # 8 y-planes per core
YH = YS // 2       # 4 rows per partition half

TAU = 20.0
REST_V = -65.0
EXC_THR = -50.0
INH_THR = -70.0
RESET_V = -65.0
ETA_LTP, ETA_LTD, WDECAY = 0.01, 0.005, 1e-05

DECAY = float(np.exp(np.float32(-1.0 / np.float32(TAU))).astype(np.float32))
ONE_MINUS_DECAY = float(np.float32(1.0) - np.float32(DECAY))
MIDPOINT = (EXC_THR + INH_THR) / 2.0  # -60.0

# slot table: (dx, dy, dz).  Halo-free (dy==0) ranges: [0:3),[5:8),[10:14),
# [22:24); each (dx,dy,dz-run) group is k-consecutive.
SLOTS = [
    (1, 0, 1), (1, 0, 0), (1, 0, -1),          # k0-2
    (1, 1, 0), (1, -1, 0),                     # k3, k4
    (-1, 0, 1), (-1, 0, 0), (-1, 0, -1),       # k5-7
    (-1, 1, 0), (-1, -1, 0),                   # k8, k9
    (0, 0, 2), (0, 0, 1),                      # k10-11
    (0, 0, -1), (0, 0, -2),                    # k12-13
    (0, 1, 1), (0, 1, 0), (0, 1, -1),          # k14-16
    (0, -1, 1), (0, -1, 0), (0, -1, -1),       # k17-19
    (0, 2, 0), (0, -2, 0),                     # k20, k21
    (2, 0, 0), (-2, 0, 0),                     # k22, k23
]
NOFF = len(SLOTS)  # 24

# mult groups: (k0, L, dy, dz_start); dz descending inside a group.
GROUPS_HF = [
    (0, 3, 0, 1), (5, 3, 0, 1),
    (10, 2, 0, 2), (12, 2, 0, -1),
    (22, 1, 0, 0), (23, 1, 0, 0),
]
GROUPS_H = [
    (3, 1, 1, 0), (4, 1, -1, 0),
    (8, 1, 1, 0), (9, 1, -1, 0),
    (14, 3, 1, 1), (17, 3, -1, 1),
    (20, 1, 2, 0), (21, 1, -2, 0),
]
GROUP_DX = {0: 1, 5: -1, 10: 0, 12: 0, 22: 2, 23: -2,
            3: 1, 4: 1, 8: -1, 9: -1, 14: 0, 17: 0, 20: 0, 21: 0}
PVX_SLICE = {1: 0, -1: 1, 2: 2, -2: 3}
SM_SLICE = {1: 0, -1: 1, 2: 2, -2: 3, 0: 4}

# PE reduction order: halo-free slots first, grouped by shift matrix.
MM_ORDER_HF = [10, 11, 12, 13, 0, 1, 2, 5, 6, 7, 22, 23]
MM_ORDER_H = [3, 4, 8, 9, 14, 15, 16, 17, 18, 19, 20, 21]

FZ = NZ + 4        # 68 field z cols
FR = 2 * YH        # 8 field rows
FFREE = FR * FZ    # 544
CH = YH * NZ       # 256 own cells per partition

F32 = mybir.dt.float32
U8 = mybir.dt.uint8

USE_F16 = bool(int(os.environ.get("BRAIN_F16", "1")))
DT = mybir.dt.float16 if USE_F16 else mybir.dt.float32
NPDT = np.float16 if USE_F16 else np.float32

_CACHE = {}


def _overlap_ap(view, kstride, ksize):
    """Insert a k dim (kstride in free elems) after the partition dim."""
    ap = [list(d) for d in view.ap]
    ap.insert(1, [kstride, ksize])
    return RawAP(tensor=view.tensor, offset=view.offset, ap=ap)


def _build_graph(nsteps):
    nc = bacc.Bacc(
        "TRN2",
        target_bir_lowering=False,
        debug=False,
        enable_asserts=True,
        num_devices=NCORES,
    )
    P128 = 2 * NX
    w0_d = nc.dram_tensor("w0", [P128, NOFF * CH], DT, kind="ExternalInput").ap()
    xin_d = nc.dram_tensor("xin", [P128, nsteps * CH], F32, kind="ExternalInput").ap()
    msk_d = nc.dram_tensor("msk", [P128, 1], F32, kind="ExternalInput").ap()
    sm_d = nc.dram_tensor("sm", [P128, 5 * P128], DT, kind="ExternalInput").ap()
    spk_d = nc.dram_tensor("spk", [nsteps, P128, CH], F32, kind="ExternalOutput").ap()

    AT = mybir.ActivationFunctionType
    ALU = mybir.AluOpType

    c = [float(np.float64(1.0 - WDECAY) ** t) for t in range(nsteps)]

    with tile.TileContext(nc) as tc, tc.tile_pool(
        name="state", bufs=1
    ) as st, tc.tile_pool(name="psum", bufs=1, space="PSUM") as ps, tc.tile_pool(
        name="dram", bufs=1, space="DRAM"
    ) as dr:
        W = st.tile([P128, NOFF * CH], DT, name="W")
        P = st.tile([P128, NOFF * CH], DT, name="P")
        FA = st.tile([P128, FFREE], DT, name="FA")
        FB = st.tile([P128, FFREE], DT, name="FB")
        QF = st.tile([P128, FFREE], DT, name="QF")
        PVX2 = [st.tile([P128, 4 * CH], DT, name=f"PVX{i}") for i in range(2)]
        SM = st.tile([P128, 5 * P128], DT, name="SM")
        XINP = st.tile([P128, nsteps * CH], F32, name="XINP")
        V = st.tile([P128, CH], F32, name="V")
        SYN = st.tile([P128, CH], F32, name="SYN")
        SS = [st.tile([P128, CH], F32, name=f"S{i}") for i in range(2)]
        SI = st.tile([P128, CH], U8, name="SI")
        II = st.tile([P128, CH], F32, name="II")
        G = st.tile([P128, CH], F32, name="G")
        E = st.tile([P128, CH], F32, name="E")
        RST = st.tile([P128, CH], F32, name="RST")
        B30 = st.tile([P128, 1], F32, name="B30")
        MSK = st.tile([P128, 1], F32, name="MSK")
        SND = st.tile([P128, 2 * NZ], DT, name="SND")
        TDP = ps.tile([P128, CH], F32, name="TDP")

        def f3(t):
            return t.rearrange("p (r z) -> p r z", z=FZ)

        def c3(t):
            return t.rearrange("p (y z) -> p y z", z=NZ)

        def w4(t, k0, L):
            return t.rearrange("p (k y z) -> p k y z", k=NOFF, z=NZ)[:, k0 : k0 + L]

        def wf(t, k0, k1):
            return t.rearrange("p (k f) -> p k f", k=NOFF)[:, k0:k1]

        def blockpair(t, r0, r1, pair0):
            v = wf(t, r0, r1)
            ap = [list(d) for d in v.ap]
            ap.insert(1, [(pair0 - r0) * CH, 2])
            return RawAP(tensor=v.tensor, offset=v.offset, ap=ap)

        def fint(t):
            return f3(t)[:, 2 : 2 + YH, 2 : 2 + NZ]

        def grp_field(t, L, dy, dz0):
            base = f3(t)[:, 2 - dy : 2 - dy + YH, 2 - dz0 : 2 - dz0 + NZ]
            if L == 1:
                return base.unsqueeze(1)
            return _overlap_ap(base, 1, L)

        def prev_bcast(pvx, t, L, dx):
            if dx == 0:
                v = fint(t)
            else:
                v = c3(pvx.rearrange("p (s f) -> p s f", s=4)[:, PVX_SLICE[dx]])
            return v.unsqueeze(1).to_broadcast([P128, L, YH, NZ])

        def xshift_dma(eng, dst, src, dx):
            for h in (0, 1):
                a, b = h * NX + max(0, dx), h * NX + NX + min(0, dx)
                eng.dma_start(dst[a:b], src[a - dx : b - dx])

        smv = SM.rearrange("p (s m) -> p s m", s=5)
        Pk = P.rearrange("p (k f) -> p k f", k=NOFF)

        def syn_mults(FPREV, groups):
            for (k0, L, dy, dz0) in groups:
                nc.vector.tensor_tensor(
                    w4(P, k0, L), w4(W, k0, L), grp_field(FPREV, L, dy, dz0),
                    ALU.mult,
                )

        def stdp_mults(FPREV, pvx, groups):
            for (k0, L, dy, dz0) in groups:
                nc.vector.tensor_tensor(
                    w4(P, k0, L), prev_bcast(pvx, FPREV, L, GROUP_DX[k0]),
                    grp_field(QF, L, dy, dz0), ALU.mult,
                )

        def w_update(ranges, hi_t):
            for (r0, r1), pair in ranges:
                if pair is not None:
                    wv = blockpair(W, r0, r1, pair)
                    pv = blockpair(P, r0, r1, pair)
                else:
                    wv, pv = wf(W, r0, r1), wf(P, r0, r1)
                nc.vector.tensor_tensor(wv, wv, pv, ALU.add)
                nc.vector.tensor_scalar(wv, wv, hi_t, 0.0, ALU.min, ALU.max)

        RANGES_HF = (((0, 3), 5), ((10, 14), None), ((22, 24), None))
        RANGES_H = (((3, 5), 8), ((14, 22), None))

        # ---- init ----
        nc.vector.memset(FA[:], 0.0)
        nc.vector.memset(FB[:], 0.0)
        nc.vector.memset(PVX2[0][:], 0.0)
        nc.vector.memset(PVX2[1][:], 0.0)
        nc.vector.memset(V[:], REST_V)
        nc.vector.memset(RST[:], RESET_V)
        nc.vector.memset(B30[:], -0.5 * MIDPOINT)
        nc.sync.dma_start(W[:], w0_d[:])
        nc.sync.dma_start(SM[:], sm_d[:])
        nc.sync.dma_start(MSK[:], msk_d[:])
        XV = XINP.rearrange("p (t f) -> p t f", t=nsteps)
        xin_v = xin_d.rearrange("p (t f) -> p t f", t=nsteps)
        nc.sync.dma_start(XV[:, 0:1], xin_v[:, 0:1])
        nc.sync.dma_start(XV[:, 1:], xin_v[:, 1:])

        pid = nc.sync.partition_id()
        offL = nc.sync.snap((pid + NCORES - 1) % NCORES, min_val=0, max_val=NCORES - 1)
        offR = nc.sync.snap((pid + 1) % NCORES, min_val=0, max_val=NCORES - 1)

        fields = [FA, FB]
        for t in range(nsteps):
            FPREV = fields[t % 2]
            FOUT = fields[(t + 1) % 2]
            last = t == nsteps - 1
            S = SS[t % 2]
            pvx = PVX2[t % 2]       # this step's stdp reads pvx
            pvx_next = PVX2[(t + 1) % 2]

            # ---- syn: halo-dependent products (HF ones were emitted in the
            # previous iteration's overlap window), PE k-reduction ----
            if t > 0:
                syn_mults(FPREV, GROUPS_H)
                order = MM_ORDER_HF + MM_ORDER_H
                for i, k in enumerate(order):
                    nc.tensor.matmul(
                        TDP[:], smv[:, SM_SLICE[SLOTS[k][0]]], Pk[:, k],
                        start=(i == 0), stop=(i == len(order) - 1),
                    )
                s_t = float(np.float32(ONE_MINUS_DECAY * c[t - 1]))
                nc.vector.scalar_tensor_tensor(
                    SYN[:], TDP[:], s_t, XV[:, t], ALU.mult, ALU.add
                )
                nc.vector.scalar_tensor_tensor(
                    V[:], V[:], DECAY, SYN[:], ALU.mult, ALU.add
                )
            else:
                nc.vector.scalar_tensor_tensor(
                    V[:], V[:], DECAY, XV[:, t], ALU.mult, ALU.add
                )

            # ---- neuron update ----
            nc.vector.tensor_scalar(S[:], V[:], EXC_THR, None, ALU.is_ge)
            nc.sync.dma_start(spk_d[t], S[:])
            if last:
                continue
            nc.vector.tensor_scalar(SI[:], V[:], EXC_THR, None, ALU.is_ge)
            nc.vector.tensor_scalar(II[:], V[:], INH_THR, None, ALU.is_le)
            nc.scalar.activation(G[:], V[:], AT.Sigmoid, bias=B30[:, 0:1], scale=0.5)
            nc.vector.tensor_tensor(E[:], S[:], II[:], ALU.subtract)
            nc.vector.tensor_tensor(E[:], G[:], E[:], ALU.add)
            nc.vector.tensor_scalar(fint(FOUT), c3(E), 1.0, 0.0, ALU.min, ALU.max)
            nc.vector.copy_predicated(V[:], SI[:], RST[:])

            # ---- boundary strips -> AllGather ----
            snd3 = SND.rearrange("p (r z) -> p r z", z=NZ)
            nc.scalar.activation(
                snd3[0:NX], f3(FOUT)[0:NX, 2:4, 2 : 2 + NZ],
                AT.Copy, bias=0.0, scale=MSK[0:NX, 0:1],
            )
            nc.scalar.activation(
                snd3[NX:P128], f3(FOUT)[NX:P128, 4:6, 2 : 2 + NZ],
                AT.Copy, bias=0.0, scale=MSK[NX:P128, 0:1],
            )
            agin = dr.tile([P128, 2 * NZ], DT, name=f"agin{t}")
            agout = dr.tile(
                [NCORES * P128, 2 * NZ], DT, addr_space="Shared", name=f"agout{t}"
            )
            nc.sync.dma_start(agin[:], SND[:])
            nc.gpsimd.collective_compute(
                "AllGather",
                ALU.bypass,
                replica_groups=[list(range(NCORES))],
                ins=[agin.opt()],
                outs=[agout.opt()],
            )
            nc.sync.dma_start(f3(FOUT)[0:NX, 6:8, 2 : 2 + NZ],
                              f3(FOUT)[NX:P128, 2:4, 2 : 2 + NZ])
            nc.sync.dma_start(f3(FOUT)[NX:P128, 0:2, 2 : 2 + NZ],
                              f3(FOUT)[0:NX, 4:6, 2 : 2 + NZ])

            # ---- overlap window: halo-free STDP + next-step prefetches ----
            do_stdp = t > 0
            if do_stdp:
                a_t = float(np.float32((ETA_LTP + ETA_LTD) / c[t]))
                b_t = float(np.float32(-ETA_LTD / c[t]))
                hi_t = float(np.float32(1.0 / c[t]))
                qf3 = f3(QF)
                fo3 = f3(FOUT)
                nc.vector.tensor_scalar(
                    qf3[:, 2:6], fo3[:, 2:6], a_t, b_t, ALU.mult, ALU.add
                )
                stdp_mults(FPREV, pvx, GROUPS_HF)
                w_update(RANGES_HF, hi_t)

            # prefetch x-shifted prev chunks for next step's stdp
            if t + 1 < nsteps - 1:
                pvs = pvx_next.rearrange("p (s f) -> p s f", s=4)
                for dxv, sl in PVX_SLICE.items():
                    xshift_dma(nc.scalar, pvs[:, sl], fint(FOUT), -dxv)

            # next step's halo-free syn products (fills the collective wait)
            if t + 1 < nsteps:
                syn_mults(FOUT, GROUPS_HF)

            # ---- halo in from neighbors ----
            agf = agout.rearrange("p (r z) -> p r z", z=NZ)
            nc.sync.dma_start(
                f3(FOUT)[0:NX, 0:2, 2 : 2 + NZ],
                agf[bass.ds(offL * P128 + NX, NX)],
            )
            nc.sync.dma_start(
                f3(FOUT)[NX:P128, 6:8, 2 : 2 + NZ],
                agf[bass.ds(offR * P128, NX)],
            )

            # ---- halo-dependent STDP ----
            if do_stdp:
                nc.vector.tensor_scalar(
                    qf3[:, 0:2], fo3[:, 0:2], a_t, b_t, ALU.mult, ALU.add
                )
                nc.vector.tensor_scalar(
                    qf3[:, 6:8], fo3[:, 6:8], a_t, b_t, ALU.mult, ALU.add
                )
                stdp_mults(FPREV, pvx, GROUPS_H)
                w_update(RANGES_H, hi_t)

    nc.compile()
    return nc


def _shard_inputs(external_input, edge_values, edge_rows, edge_cols, nsteps):
    ext = np.ascontiguousarray(np.asarray(external_input, dtype=np.float32))[:nsteps]
    vals = np.asarray(edge_values, dtype=np.float32)
    rows = np.asarray(edge_rows, dtype=np.int64)
    cols = np.asarray(edge_cols, dtype=np.int64)

    dlin = cols - rows
    offs_lin = np.array([d[0] * NY * NZ + d[1] * NZ + d[2] for d in SLOTS])
    k_of = {int(v): i for i, v in enumerate(offs_lin)}
    ke = np.array([k_of[int(v)] for v in dlin], dtype=np.int64)
    Wd = np.zeros((NOFF, N), dtype=np.float32)
    Wd[ke, cols] = vals
    Wd = Wd.reshape(NOFF, NX, NY, NZ)

    Wsrc = np.zeros_like(Wd)
    for k, (dx, _, _) in enumerate(SLOTS):
        if dx >= 0:
            Wsrc[k, : NX - dx] = Wd[k, dx:]
        else:
            Wsrc[k, -dx:] = Wd[k, : NX + dx]

    ext4 = ext.reshape(nsteps, NX, NY, NZ) * np.float32(ONE_MINUS_DECAY)

    # shift matrices: slices (+1,-1,+2,-2,identity); SM_s[p,m]=1 iff m=p+dx
    sm = np.zeros((2 * NX, 5, 2 * NX), dtype=np.float32)
    for s, dxv in enumerate((1, -1, 2, -2, 0)):
        for h in (0, 1):
            for xs in range(NX):
                xm = xs + dxv
                if 0 <= xm < NX:
                    sm[h * NX + xs, s, h * NX + xm] = 1.0
    sm = sm.reshape(2 * NX, 5 * 2 * NX).astype(NPDT)

    in_maps = []
    for cidx in range(NCORES):
        ylo = cidx * YS
        sub = Wsrc[:, :, ylo : ylo + YS, :]
        tr = sub.transpose(1, 0, 2, 3)
        wc = np.concatenate(
            [tr[:, :, :YH, :].reshape(NX, NOFF * CH),
             tr[:, :, YH:, :].reshape(NX, NOFF * CH)], axis=0
        ).astype(NPDT)
        esub = ext4[:, :, ylo : ylo + YS, :].transpose(1, 0, 2, 3)
        xc = np.concatenate(
            [esub[:, :, :YH, :].reshape(NX, nsteps * CH),
             esub[:, :, YH:, :].reshape(NX, nsteps * CH)], axis=0
        ).astype(np.float32)
        msk = np.ones((2 * NX, 1), dtype=np.float32)
        if cidx == 0:
            msk[:NX] = 0.0
        if cidx == NCORES - 1:
            msk[NX:] = 0.0
        in_maps.append(
            {"w0": np.ascontiguousarray(wc), "xin": np.ascontiguousarray(xc),
             "msk": msk, "sm": sm}
        )
    return in_maps


def kernel(external_input, edge_values, edge_rows, edge_cols, num_steps):
    nsteps = int(num_steps)
    if nsteps not in _CACHE:
        _CACHE[nsteps] = _build_graph(nsteps)
    nc = _CACHE[nsteps]

    in_maps = _shard_inputs(external_input, edge_values, edge_rows, edge_cols, nsteps)
    res = bass_utils.run_bass_kernel_spmd(
        nc,
        in_maps,
        core_ids=list(range(NCORES)),
        trace=bool(int(os.environ.get("BRAIN_TRACE", "0"))),
    )

    out = np.empty((nsteps, NX, NY, NZ), dtype=np.float32)
    for cidx in range(NCORES):
        ylo = cidx * YS
        spk = res.results[cidx]["spk"].reshape(nsteps, 2, NX, YH, NZ)
        out[:, :, ylo : ylo + YH, :] = spk[:, 0]
        out[:, :, ylo + YH : ylo + YS, :] = spk[:, 1]
    kernel.last_results = res
    return out.reshape(nsteps, N)


# revision 13
# speedup vs baseline: 4.6299x; 1.0251x over previous
"""Trainium2 Bass kernel for Brain3DQTUNNetwork (gnn_message_passing), v3.

Per core: y-slab of 8 planes, 128 partitions p = h*64 + x (h = y-half).
Weights stored SOURCE-x-aligned per offset slot, so the SpMV reads the prev
field with only (dy,dz) free-dim shifts; per-slot products are realigned to
destinations and k-reduced by 24 tiny PE matmuls (banded 0/1 shift matrices
/ identity) accumulating in PSUM.  Products/weights run fp16 (DVE 2x_1p /
4x_2p); V/neuron path fp32.  Weight decay is folded into a global scale
c_t = (1-wd)^t so the STDP update is a plain fp16 add + 2-op clip.
Cross-core traffic: one AllGather of 2-row boundary strips per step,
overlapped with halo-free STDP and the NEXT step's halo-free syn products
(software pipelining).
"""

import os
import sys

sys.path.insert(0, "/opt/trn_rl_repo")

import numpy as np

import concourse.bass as bass
import concourse.bacc as bacc
import concourse.mybir as mybir
import concourse.tile as tile
from concourse import bass_utils
from bass_rust import AP as RawAP

# ---- problem constants (hardcoded; kernel.py must be self-contained) ----
GRID = (64, 64, 64)
NX, NY, NZ = GRID
N = NX * NY * NZ
RADIUS = 2
NCORES = 8
YS = NY // NCORES  # 8 y-planes per core
YH = YS // 2       # 4 rows per partition half

TAU = 20.0
REST_V = -65.0
EXC_THR = -50.0
INH_THR = -70.0
RESET_V = -65.0
ETA_LTP, ETA_LTD, WDECAY = 0.01, 0.005, 1e-05

DECAY = float(np.exp(np.float32(-1.0 / np.float32(TAU))).astype(np.float32))
ONE_MINUS_DECAY = float(np.float32(1.0) - np.float32(DECAY))
MIDPOINT = (EXC_THR + INH_THR) / 2.0  # -60.0

# slot table: (dx, dy, dz).  Halo-free (dy==0) ranges: [0:3),[5:8),[10:14),
# [22:24); each (dx,dy,dz-run) group is k-consecutive.
SLOTS = [
    (1, 0, 1), (1, 0, 0), (1, 0, -1),          # k0-2
    (1, 1, 0), (1, -1, 0),                     # k3, k4
    (-1, 0, 1), (-1, 0, 0), (-1, 0, -1),       # k5-7
    (-1, 1, 0), (-1, -1, 0),                   # k8, k9
    (0, 0, 2), (0, 0, 1),                      # k10-11
    (0, 0, -1), (0, 0, -2),                    # k12-13
    (0, 1, 1), (0, 1, 0), (0, 1, -1),          # k14-16
    (0, -1, 1), (0, -1, 0), (0, -1, -1),       # k17-19
    (0, 2, 0), (0, -2, 0),                     # k20, k21
    (2, 0, 0), (-2, 0, 0),                     # k22, k23
]
NOFF = len(SLOTS)  # 24

# mult groups: (k0, L, dy, dz_start); dz descending inside a group.
GROUPS_HF = [
    (0, 3, 0, 1), (5, 3, 0, 1),
    (10, 2, 0, 2), (12, 2, 0, -1),
    (22, 1, 0, 0), (23, 1, 0, 0),
]
GROUPS_H = [
    (14, 3, 1, 1), (17, 3, -1, 1),
    (3, 1, 1, 0), (4, 1, -1, 0),
    (8, 1, 1, 0), (9, 1, -1, 0),
    (20, 1, 2, 0), (21, 1, -2, 0),
]
GROUP_DX = {0: 1, 5: -1, 10: 0, 12: 0, 22: 2, 23: -2,
            3: 1, 4: 1, 8: -1, 9: -1, 14: 0, 17: 0, 20: 0, 21: 0}
PVX_SLICE = {1: 0, -1: 1, 2: 2, -2: 3}
SM_SLICE = {1: 0, -1: 1, 2: 2, -2: 3, 0: 4}

# PE reduction order: halo-free slots first, grouped by shift matrix.
MM_ORDER_HF = [10, 11, 12, 13, 0, 1, 2, 5, 6, 7, 22, 23]
MM_ORDER_H = [14, 15, 16, 17, 18, 19, 3, 4, 8, 9, 20, 21]

FZ = NZ + 4        # 68 field z cols
FR = 2 * YH        # 8 field rows
FFREE = FR * FZ    # 544
CH = YH * NZ       # 256 own cells per partition

F32 = mybir.dt.float32
U8 = mybir.dt.uint8

USE_F16 = bool(int(os.environ.get("BRAIN_F16", "1")))
DT = mybir.dt.float16 if USE_F16 else mybir.dt.float32
NPDT = np.float16 if USE_F16 else np.float32

_CACHE = {}


def _overlap_ap(view, kstride, ksize):
    """Insert a k dim (kstride in free elems) after the partition dim."""
    ap = [list(d) for d in view.ap]
    ap.insert(1, [kstride, ksize])
    return RawAP(tensor=view.tensor, offset=view.offset, ap=ap)


def _build_graph(nsteps):
    nc = bacc.Bacc(
        "TRN2",
        target_bir_lowering=False,
        debug=False,
        enable_asserts=True,
        num_devices=NCORES,
    )
    P128 = 2 * NX
    w0_d = nc.dram_tensor("w0", [P128, NOFF * CH], DT, kind="ExternalInput").ap()
    xin_d = nc.dram_tensor("xin", [P128, nsteps * CH], F32, kind="ExternalInput").ap()
    msk_d = nc.dram_tensor("msk", [P128, 2], F32, kind="ExternalInput").ap()
    sm_d = nc.dram_tensor("sm", [P128, 5 * P128], DT, kind="ExternalInput").ap()
    spk_d = nc.dram_tensor("spk", [nsteps, P128, CH], F32, kind="ExternalOutput").ap()

    AT = mybir.ActivationFunctionType
    ALU = mybir.AluOpType

    c = [float(np.float64(1.0 - WDECAY) ** t) for t in range(nsteps)]

    with tile.TileContext(nc) as tc, tc.tile_pool(
        name="state", bufs=1
    ) as st, tc.tile_pool(name="psum", bufs=1, space="PSUM") as ps, tc.tile_pool(
        name="dram", bufs=1, space="DRAM"
    ) as dr:
        W = st.tile([P128, NOFF * CH], DT, name="W")
        P = st.tile([P128, NOFF * CH], DT, name="P")
        FA = st.tile([P128, FFREE], DT, name="FA")
        FB = st.tile([P128, FFREE], DT, name="FB")
        QF = st.tile([P128, FFREE], DT, name="QF")
        PVX2 = [st.tile([P128, 4 * CH], DT, name=f"PVX{i}") for i in range(2)]
        SM = st.tile([P128, 5 * P128], DT, name="SM")
        XINP = st.tile([P128, nsteps * CH], F32, name="XINP")
        V = st.tile([P128, CH], F32, name="V")
        SYN = st.tile([P128, CH], F32, name="SYN")
        SS = [st.tile([P128, CH], F32, name=f"S{i}") for i in range(2)]
        SI = st.tile([P128, CH], U8, name="SI")
        II = st.tile([P128, CH], F32, name="II")
        G = st.tile([P128, CH], F32, name="G")
        E = st.tile([P128, CH], F32, name="E")
        RST = st.tile([P128, CH], F32, name="RST")
        B30 = st.tile([P128, 1], F32, name="B30")
        MSKB = st.tile([P128, 2], F32, name="MSKB")
        SA = st.tile([P128, 2], F32, name="SA")
        TDP = ps.tile([P128, CH], F32, name="TDP")

        def f3(t):
            return t.rearrange("p (r z) -> p r z", z=FZ)

        def c3(t):
            return t.rearrange("p (y z) -> p y z", z=NZ)

        def w4(t, k0, L):
            return t.rearrange("p (k y z) -> p k y z", k=NOFF, z=NZ)[:, k0 : k0 + L]

        def wf(t, k0, k1):
            return t.rearrange("p (k f) -> p k f", k=NOFF)[:, k0:k1]

        def blockpair(t, r0, r1, pair0):
            v = wf(t, r0, r1)
            ap = [list(d) for d in v.ap]
            ap.insert(1, [(pair0 - r0) * CH, 2])
            return RawAP(tensor=v.tensor, offset=v.offset, ap=ap)

        def fint(t):
            return f3(t)[:, 2 : 2 + YH, 2 : 2 + NZ]

        def grp_field(t, L, dy, dz0):
            base = f3(t)[:, 2 - dy : 2 - dy + YH, 2 - dz0 : 2 - dz0 + NZ]
            if L == 1:
                return base.unsqueeze(1)
            return _overlap_ap(base, 1, L)

        def prev_bcast(pvx, t, L, dx):
            if dx == 0:
                v = fint(t)
            else:
                v = c3(pvx.rearrange("p (s f) -> p s f", s=4)[:, PVX_SLICE[dx]])
            return v.unsqueeze(1).to_broadcast([P128, L, YH, NZ])

        def xshift_dma(eng, dst, src, dx):
            for h in (0, 1):
                a, b = h * NX + max(0, dx), h * NX + NX + min(0, dx)
                eng.dma_start(dst[a:b], src[a - dx : b - dx])

        smv = SM.rearrange("p (s m) -> p s m", s=5)
        Pk = P.rearrange("p (k f) -> p k f", k=NOFF)

        def syn_mults(FPREV, groups):
            for (k0, L, dy, dz0) in groups:
                nc.vector.tensor_tensor(
                    w4(P, k0, L), w4(W, k0, L), grp_field(FPREV, L, dy, dz0),
                    ALU.mult,
                )

        def stdp_mults(FPREV, pvx, groups):
            for (k0, L, dy, dz0) in groups:
                nc.vector.tensor_tensor(
                    w4(P, k0, L), prev_bcast(pvx, FPREV, L, GROUP_DX[k0]),
                    grp_field(QF, L, dy, dz0), ALU.mult,
                )

        def w_update(ranges, hi_t):
            for (r0, r1), pair in ranges:
                if pair is not None:
                    wv = blockpair(W, r0, r1, pair)
                    pv = blockpair(P, r0, r1, pair)
                else:
                    wv, pv = wf(W, r0, r1), wf(P, r0, r1)
                nc.vector.tensor_tensor(wv, wv, pv, ALU.add)
                nc.vector.tensor_scalar(wv, wv, hi_t, 0.0, ALU.min, ALU.max)

        RANGES_HF = (((0, 3), 5), ((10, 14), None), ((22, 24), None))

        # ---- init ----
        nc.vector.memset(FA[:], 0.0)
        nc.vector.memset(FB[:], 0.0)
        nc.vector.memset(PVX2[0][:], 0.0)
        nc.vector.memset(PVX2[1][:], 0.0)
        nc.vector.memset(V[:], REST_V)
        nc.vector.memset(RST[:], RESET_V)
        nc.vector.memset(B30[:], -0.5 * MIDPOINT)
        nc.sync.dma_start(W[:], w0_d[:])
        nc.sync.dma_start(SM[:], sm_d[:])
        nc.sync.dma_start(MSKB[:], msk_d[:])
        XV = XINP.rearrange("p (t f) -> p t f", t=nsteps)
        xin_v = xin_d.rearrange("p (t f) -> p t f", t=nsteps)
        nc.sync.dma_start(XV[:, 0:1], xin_v[:, 0:1])
        nc.sync.dma_start(XV[:, 1:], xin_v[:, 1:])

        pid = nc.sync.partition_id()
        offL = nc.sync.snap((pid + NCORES - 1) % NCORES, min_val=0, max_val=NCORES - 1)
        offR = nc.sync.snap((pid + 1) % NCORES, min_val=0, max_val=NCORES - 1)

        fields = [FA, FB]
        for t in range(nsteps):
            FPREV = fields[t % 2]
            FOUT = fields[(t + 1) % 2]
            last = t == nsteps - 1
            S = SS[t % 2]
            pvx = PVX2[t % 2]       # this step's stdp reads pvx
            pvx_next = PVX2[(t + 1) % 2]

            # ---- syn: halo-dependent products (HF ones were emitted in the
            # previous iteration's overlap window), PE k-reduction ----
            if t > 0:
                syn_mults(FPREV, GROUPS_H)
                order = MM_ORDER_HF + MM_ORDER_H
                for i, k in enumerate(order):
                    nc.tensor.matmul(
                        TDP[:], smv[:, SM_SLICE[SLOTS[k][0]]], Pk[:, k],
                        start=(i == 0), stop=(i == len(order) - 1),
                    )
                s_t = float(np.float32(ONE_MINUS_DECAY * c[t - 1]))
                nc.vector.scalar_tensor_tensor(
                    SYN[:], TDP[:], s_t, XV[:, t], ALU.mult, ALU.add
                )
                nc.vector.scalar_tensor_tensor(
                    V[:], V[:], DECAY, SYN[:], ALU.mult, ALU.add
                )
            else:
                nc.vector.scalar_tensor_tensor(
                    V[:], V[:], DECAY, XV[:, t], ALU.mult, ALU.add
                )

            # ---- neuron update ----
            nc.vector.tensor_scalar(S[:], V[:], EXC_THR, None, ALU.is_ge)
            nc.sync.dma_start(spk_d[t], S[:])
            if last:
                continue
            nc.vector.tensor_scalar(SI[:], V[:], EXC_THR, None, ALU.is_ge)
            nc.vector.tensor_scalar(II[:], V[:], INH_THR, None, ALU.is_le)
            nc.scalar.activation(G[:], V[:], AT.Sigmoid, bias=B30[:, 0:1], scale=0.5)
            nc.vector.tensor_tensor(E[:], S[:], II[:], ALU.subtract)
            nc.vector.tensor_tensor(E[:], G[:], E[:], ALU.add)
            nc.vector.tensor_scalar(fint(FOUT), c3(E), 1.0, 0.0, ALU.min, ALU.max)
            nc.vector.copy_predicated(V[:], SI[:], RST[:])

            # ---- boundary strips -> AllGather (unmasked; edge-core wrap
            # garbage is neutralized by W'=0 on syn and the masked QF-halo
            # scale on STDP) ----
            agin = dr.tile([P128, 2 * NZ], DT, name=f"agin{t}")
            agout = dr.tile(
                [NCORES * P128, 2 * NZ], DT, addr_space="Shared", name=f"agout{t}"
            )
            agv = agin.rearrange("p (r z) -> p r z", z=NZ)
            nc.sync.dma_start(agv[0:NX], f3(FOUT)[0:NX, 2:4, 2 : 2 + NZ])
            nc.sync.dma_start(agv[NX:P128], f3(FOUT)[NX:P128, 4:6, 2 : 2 + NZ])
            nc.gpsimd.collective_compute(
                "AllGather",
                ALU.bypass,
                replica_groups=[list(range(NCORES))],
                ins=[agin.opt()],
                outs=[agout.opt()],
            )
            nc.sync.dma_start(f3(FOUT)[0:NX, 6:8, 2 : 2 + NZ],
                              f3(FOUT)[NX:P128, 2:4, 2 : 2 + NZ])
            nc.sync.dma_start(f3(FOUT)[NX:P128, 0:2, 2 : 2 + NZ],
                              f3(FOUT)[0:NX, 4:6, 2 : 2 + NZ])

            # ---- overlap window: halo-free STDP + next-step prefetches ----
            do_stdp = t > 0
            if do_stdp:
                a_t = float(np.float32((ETA_LTP + ETA_LTD) / c[t]))
                b_t = float(np.float32(-ETA_LTD / c[t]))
                hi_t = float(np.float32(1.0 / c[t]))
                qf3 = f3(QF)
                fo3 = f3(FOUT)
                nc.vector.tensor_scalar(
                    qf3[:, 2:6], fo3[:, 2:6], a_t, b_t, ALU.mult, ALU.add
                )
                # per-step masked QF-halo scales (zero invalid halo sources)
                nc.vector.tensor_scalar(SA[:], MSKB[:], a_t, None, ALU.mult)
                stdp_mults(FPREV, pvx, GROUPS_HF)
                w_update(RANGES_HF, hi_t)

            # prefetch x-shifted prev chunks for next step's stdp
            if t + 1 < nsteps - 1:
                pvs = pvx_next.rearrange("p (s f) -> p s f", s=4)
                for dxv, sl in PVX_SLICE.items():
                    xshift_dma(nc.scalar, pvs[:, sl], fint(FOUT), -dxv)

            # next step's halo-free syn products (fills the collective wait)
            if t + 1 < nsteps:
                syn_mults(FOUT, GROUPS_HF)

            # ---- halo in from neighbors ----
            agf = agout.rearrange("p (r z) -> p r z", z=NZ)
            nc.sync.dma_start(
                f3(FOUT)[0:NX, 0:2, 2 : 2 + NZ],
                agf[bass.ds(offL * P128 + NX, NX)],
            )
            nc.sync.dma_start(
                f3(FOUT)[NX:P128, 6:8, 2 : 2 + NZ],
                agf[bass.ds(offR * P128, NX)],
            )

            # ---- halo-dependent STDP (interleaved mult/update for
            # per-range pipelining behind the two halo-in DMAs) ----
            if do_stdp:
                nc.vector.tensor_scalar(
                    qf3[:, 0:2], fo3[:, 0:2], SA[:, 0:1], b_t, ALU.mult, ALU.add
                )
                nc.vector.tensor_scalar(
                    qf3[:, 6:8], fo3[:, 6:8], SA[:, 1:2], b_t, ALU.mult, ALU.add
                )
                stdp_mults(FPREV, pvx, [(14, 3, 1, 1)])
                w_update((((14, 17), None),), hi_t)
                stdp_mults(FPREV, pvx, [(17, 3, -1, 1)])
                w_update((((17, 20), None),), hi_t)
                stdp_mults(FPREV, pvx,
                           [(3, 1, 1, 0), (4, 1, -1, 0),
                            (8, 1, 1, 0), (9, 1, -1, 0)])
                w_update((((3, 5), 8),), hi_t)
                stdp_mults(FPREV, pvx, [(20, 1, 2, 0), (21, 1, -2, 0)])
                w_update((((20, 22), None),), hi_t)

    nc.compile()
    return nc


def _shard_inputs(external_input, edge_values, edge_rows, edge_cols, nsteps):
    ext = np.ascontiguousarray(np.asarray(external_input, dtype=np.float32))[:nsteps]
    vals = np.asarray(edge_values, dtype=np.float32)
    rows = np.asarray(edge_rows, dtype=np.int64)
    cols = np.asarray(edge_cols, dtype=np.int64)

    dlin = cols - rows
    offs_lin = np.array([d[0] * NY * NZ + d[1] * NZ + d[2] for d in SLOTS])
    k_of = {int(v): i for i, v in enumerate(offs_lin)}
    ke = np.array([k_of[int(v)] for v in dlin], dtype=np.int64)
    Wd = np.zeros((NOFF, N), dtype=np.float32)
    Wd[ke, cols] = vals
    Wd = Wd.reshape(NOFF, NX, NY, NZ)

    Wsrc = np.zeros_like(Wd)
    for k, (dx, _, _) in enumerate(SLOTS):
        if dx >= 0:
            Wsrc[k, : NX - dx] = Wd[k, dx:]
        else:
            Wsrc[k, -dx:] = Wd[k, : NX + dx]

    ext4 = ext.reshape(nsteps, NX, NY, NZ) * np.float32(ONE_MINUS_DECAY)

    # shift matrices: slices (+1,-1,+2,-2,identity); SM_s[p,m]=1 iff m=p+dx
    sm = np.zeros((2 * NX, 5, 2 * NX), dtype=np.float32)
    for s, dxv in enumerate((1, -1, 2, -2, 0)):
        for h in (0, 1):
            for xs in range(NX):
                xm = xs + dxv
                if 0 <= xm < NX:
                    sm[h * NX + xs, s, h * NX + xm] = 1.0
    sm = sm.reshape(2 * NX, 5 * 2 * NX).astype(NPDT)

    in_maps = []
    for cidx in range(NCORES):
        ylo = cidx * YS
        sub = Wsrc[:, :, ylo : ylo + YS, :]
        tr = sub.transpose(1, 0, 2, 3)
        wc = np.concatenate(
            [tr[:, :, :YH, :].reshape(NX, NOFF * CH),
             tr[:, :, YH:, :].reshape(NX, NOFF * CH)], axis=0
        ).astype(NPDT)
        esub = ext4[:, :, ylo : ylo + YS, :].transpose(1, 0, 2, 3)
        xc = np.concatenate(
            [esub[:, :, :YH, :].reshape(NX, nsteps * CH),
             esub[:, :, YH:, :].reshape(NX, nsteps * CH)], axis=0
        ).astype(np.float32)
        # halo-validity masks: col 0 gates field rows 0:2 (h0 <- left
        # neighbor, h1 <- intra); col 1 gates rows 6:8 (h0 <- intra,
        # h1 <- right neighbor)
        msk = np.ones((2 * NX, 2), dtype=np.float32)
        if cidx == 0:
            msk[:NX, 0] = 0.0
        if cidx == NCORES - 1:
            msk[NX:, 1] = 0.0
        in_maps.append(
            {"w0": np.ascontiguousarray(wc), "xin": np.ascontiguousarray(xc),
             "msk": msk, "sm": sm}
        )
    return in_maps


def kernel(external_input, edge_values, edge_rows, edge_cols, num_steps):
    nsteps = int(num_steps)
    if nsteps not in _CACHE:
        _CACHE[nsteps] = _build_graph(nsteps)
    nc = _CACHE[nsteps]

    in_maps = _shard_inputs(external_input, edge_values, edge_rows, edge_cols, nsteps)
    res = bass_utils.run_bass_kernel_spmd(
        nc,
        in_maps,
        core_ids=list(range(NCORES)),
        trace=bool(int(os.environ.get("BRAIN_TRACE", "0"))),
    )

    out = np.empty((nsteps, NX, NY, NZ), dtype=np.float32)
    for cidx in range(NCORES):
        ylo = cidx * YS
        spk = res.results[cidx]["spk"].reshape(nsteps, 2, NX, YH, NZ)
        out[:, :, ylo : ylo + YH, :] = spk[:, 0]
        out[:, :, ylo + YH : ylo + YS, :] = spk[:, 1]
    kernel.last_results = res
    return out.reshape(nsteps, N)


# revision 19
# speedup vs baseline: 4.7573x; 1.0275x over previous
"""Trainium2 Bass kernel for Brain3DQTUNNetwork (gnn_message_passing), v3.

Per core: y-slab of 8 planes, 128 partitions p = h*64 + x (h = y-half).
Weights stored SOURCE-x-aligned per offset slot, so the SpMV reads the prev
field with only (dy,dz) free-dim shifts; per-slot products are realigned to
destinations and k-reduced by 24 tiny PE matmuls (banded 0/1 shift matrices
/ identity) accumulating in PSUM.  Products/weights run fp16 (DVE 2x_1p /
4x_2p); V/neuron path fp32.  Weight decay is folded into a global scale
c_t = (1-wd)^t so the STDP update is a plain fp16 add + 2-op clip.
Cross-core traffic: one AllGather of 2-row boundary strips per step,
overlapped with halo-free STDP and the NEXT step's halo-free syn products
(software pipelining).
"""

import os
import sys

sys.path.insert(0, "/opt/trn_rl_repo")

import numpy as np

import concourse.bass as bass
import concourse.bacc as bacc
import concourse.mybir as mybir
import concourse.tile as tile
from concourse import bass_utils
from bass_rust import AP as RawAP

# ---- problem constants (hardcoded; kernel.py must be self-contained) ----
GRID = (64, 64, 64)
NX, NY, NZ = GRID
N = NX * NY * NZ
RADIUS = 2
NCORES = 8
YS = NY // NCORES  # 8 y-planes per core
YH = YS // 2       # 4 rows per partition half

TAU = 20.0
REST_V = -65.0
EXC_THR = -50.0
INH_THR = -70.0
RESET_V = -65.0
ETA_LTP, ETA_LTD, WDECAY = 0.01, 0.005, 1e-05

DECAY = float(np.exp(np.float32(-1.0 / np.float32(TAU))).astype(np.float32))
ONE_MINUS_DECAY = float(np.float32(1.0) - np.float32(DECAY))
MIDPOINT = (EXC_THR + INH_THR) / 2.0  # -60.0

# slot table: (dx, dy, dz).  Halo-free (dy==0) ranges: [0:3),[5:8),[10:14),
# [22:24); each (dx,dy,dz-run) group is k-consecutive.
SLOTS = [
    (1, 0, 1), (1, 0, 0), (1, 0, -1),          # k0-2
    (1, 1, 0), (1, -1, 0),                     # k3, k4
    (-1, 0, 1), (-1, 0, 0), (-1, 0, -1),       # k5-7
    (-1, 1, 0), (-1, -1, 0),                   # k8, k9
    (0, 0, 2), (0, 0, 1),                      # k10-11
    (0, 0, -1), (0, 0, -2),                    # k12-13
    (0, 1, 1), (0, 1, 0), (0, 1, -1),          # k14-16
    (0, -1, 1), (0, -1, 0), (0, -1, -1),       # k17-19
    (0, 2, 0), (0, -2, 0),                     # k20, k21
    (2, 0, 0), (-2, 0, 0),                     # k22, k23
]
NOFF = len(SLOTS)  # 24

# mult groups.  run: (k0, L, dy, dz_start) with dz descending (field
# k-stride +1).  pair: (ka, kb, dy, dz0, fstride, prev) — two slots sharing
# one instruction, field k-stride `fstride` (0 = broadcast), prev side
# `prev` in {0 (FPREV), 'p01', 'p23'} (PVX slice pairs).
GROUPS_HF = [
    ("run", 0, 3, 0, 1, 1), ("run", 5, 3, 0, 1, -1),
    ("run", 10, 2, 0, 2, 0), ("run", 12, 2, 0, -1, 0),
    ("pair", 22, 23, 0, 0, 0, "p23"),
]
GROUPS_H = [
    ("run", 14, 3, 1, 1, 0), ("run", 17, 3, -1, 1, 0),
    ("pair", 3, 8, 1, 0, 0, "p01"),
    ("pair", 4, 9, -1, 0, 0, "p01"),
    ("pair", 20, 21, 2, 0, 4 * 68, 0),  # dy=+2 base; +272 elems = dy=-2 rows
]
PVX_OF_K = {0: 1, 5: -1, 22: 2, 23: -2, 3: 1, 4: 1, 8: -1, 9: -1}
PVX_SLICE = {1: 0, -1: 1, 2: 2, -2: 3}
SM_SLICE = {1: 0, -1: 1, 2: 2, -2: 3, 0: 4}

# PE reduction order: halo-free slots first, grouped by shift matrix.
MM_ORDER_HF = [10, 11, 12, 13, 0, 1, 2, 5, 6, 7, 22, 23]
MM_ORDER_H = [14, 15, 16, 17, 18, 19, 3, 4, 8, 9, 20, 21]

FZ = NZ + 4        # 68 field z cols
FR = 2 * YH        # 8 field rows
FFREE = FR * FZ    # 544
CH = YH * NZ       # 256 own cells per partition

F32 = mybir.dt.float32
U8 = mybir.dt.uint8

USE_F16 = bool(int(os.environ.get("BRAIN_F16", "1")))
DT = mybir.dt.float16 if USE_F16 else mybir.dt.float32
NPDT = np.float16 if USE_F16 else np.float32

_CACHE = {}


def _overlap_ap(view, kstride, ksize):
    """Insert a k dim (kstride in free elems) after the partition dim."""
    ap = [list(d) for d in view.ap]
    ap.insert(1, [kstride, ksize])
    return RawAP(tensor=view.tensor, offset=view.offset, ap=ap)


def _build_graph(nsteps):
    nc = bacc.Bacc(
        "TRN2",
        target_bir_lowering=False,
        debug=False,
        enable_asserts=True,
        num_devices=NCORES,
    )
    P128 = 2 * NX
    w0_d = nc.dram_tensor("w0", [P128, NOFF * CH], DT, kind="ExternalInput").ap()
    xin_d = nc.dram_tensor("xin", [P128, nsteps * CH], F32, kind="ExternalInput").ap()
    msk_d = nc.dram_tensor("msk", [P128, 2], F32, kind="ExternalInput").ap()
    sm_d = nc.dram_tensor("sm", [P128, 5 * P128], DT, kind="ExternalInput").ap()
    spk_d = nc.dram_tensor("spk", [nsteps, P128, CH], F32, kind="ExternalOutput").ap()

    AT = mybir.ActivationFunctionType
    ALU = mybir.AluOpType

    c = [float(np.float64(1.0 - WDECAY) ** t) for t in range(nsteps)]

    with tile.TileContext(nc) as tc, tc.tile_pool(
        name="state", bufs=1
    ) as st, tc.tile_pool(name="psum", bufs=1, space="PSUM") as ps, tc.tile_pool(
        name="dram", bufs=1, space="DRAM"
    ) as dr:
        W = st.tile([P128, NOFF * CH], DT, name="W")
        P = st.tile([P128, NOFF * CH], DT, name="P")
        FA = st.tile([P128, FFREE], DT, name="FA")
        FB = st.tile([P128, FFREE], DT, name="FB")
        QF = st.tile([P128, FFREE], DT, name="QF")
        PVX2 = [st.tile([P128, 4 * CH], DT, name=f"PVX{i}") for i in range(2)]
        SM = st.tile([P128, 5 * P128], DT, name="SM")
        XINP = st.tile([P128, nsteps * CH], F32, name="XINP")
        V = st.tile([P128, CH], F32, name="V")
        SYN = st.tile([P128, CH], F32, name="SYN")
        SS = [st.tile([P128, CH], F32, name=f"S{i}") for i in range(2)]
        SI = st.tile([P128, CH], U8, name="SI")
        II = st.tile([P128, CH], F32, name="II")
        G = st.tile([P128, CH], F32, name="G")
        E = st.tile([P128, CH], F32, name="E")
        RST = st.tile([P128, CH], F32, name="RST")
        B30 = st.tile([P128, 1], F32, name="B30")
        MSKB = st.tile([P128, 2], F32, name="MSKB")
        SA = st.tile([P128, 2], F32, name="SA")
        TDP = ps.tile([P128, CH], F32, name="TDP")

        def f3(t):
            return t.rearrange("p (r z) -> p r z", z=FZ)

        def c3(t):
            return t.rearrange("p (y z) -> p y z", z=NZ)

        def w4(t, k0, L):
            return t.rearrange("p (k y z) -> p k y z", k=NOFF, z=NZ)[:, k0 : k0 + L]

        def wf(t, k0, k1):
            return t.rearrange("p (k f) -> p k f", k=NOFF)[:, k0:k1]

        def blockpair(t, r0, r1, pair0):
            v = wf(t, r0, r1)
            ap = [list(d) for d in v.ap]
            ap.insert(1, [(pair0 - r0) * CH, 2])
            return RawAP(tensor=v.tensor, offset=v.offset, ap=ap)

        def fint(t):
            return f3(t)[:, 2 : 2 + YH, 2 : 2 + NZ]

        def grp_field(t, L, dy, dz0, kstride=1):
            base = f3(t)[:, 2 - dy : 2 - dy + YH, 2 - dz0 : 2 - dz0 + NZ]
            if L == 1:
                return base.unsqueeze(1)
            return _overlap_ap(base, kstride, L)

        def pairk(t, ka, kb):
            v = w4(t, ka, 1)
            ap = [list(d) for d in v.ap]
            ap[1] = [(kb - ka) * CH, 2]
            return RawAP(tensor=v.tensor, offset=v.offset, ap=ap)

        def prev_bcast(pvx, t, L, dx):
            if dx == 0:
                v = fint(t)
            else:
                v = c3(pvx.rearrange("p (s f) -> p s f", s=4)[:, PVX_SLICE[dx]])
            return v.unsqueeze(1).to_broadcast([P128, L, YH, NZ])

        def pvx4(pvx):
            return pvx.rearrange("p (s y z) -> p s y z", s=4, z=NZ)

        def xshift_dma(eng, dst, src, dx):
            for h in (0, 1):
                a, b = h * NX + max(0, dx), h * NX + NX + min(0, dx)
                eng.dma_start(dst[a:b], src[a - dx : b - dx])

        smv = SM.rearrange("p (s m) -> p s m", s=5)
        Pk = P.rearrange("p (k f) -> p k f", k=NOFF)

        def syn_mults(FPREV, groups):
            for g in groups:
                if g[0] == "run":
                    _, k0, L, dy, dz0, _ = g
                    nc.vector.tensor_tensor(
                        w4(P, k0, L), w4(W, k0, L), grp_field(FPREV, L, dy, dz0),
                        ALU.mult,
                    )
                else:
                    _, ka, kb, dy, dz0, fs, _ = g
                    nc.vector.tensor_tensor(
                        pairk(P, ka, kb), pairk(W, ka, kb),
                        grp_field(FPREV, 2, dy, dz0, kstride=fs),
                        ALU.mult,
                    )

        def stdp_mults(FPREV, pvx, groups):
            for g in groups:
                if g[0] == "run":
                    _, k0, L, dy, dz0, pdx = g
                    nc.vector.tensor_tensor(
                        w4(P, k0, L), prev_bcast(pvx, FPREV, L, pdx),
                        grp_field(QF, L, dy, dz0), ALU.mult,
                    )
                else:
                    _, ka, kb, dy, dz0, fs, prev = g
                    if prev == "p01":
                        pv = pvx4(pvx)[:, 0:2]
                    elif prev == "p23":
                        pv = pvx4(pvx)[:, 2:4]
                    else:
                        pv = fint(FPREV).unsqueeze(1).to_broadcast(
                            [P128, 2, YH, NZ]
                        )
                    nc.vector.tensor_tensor(
                        pairk(P, ka, kb), pv,
                        grp_field(QF, 2, dy, dz0, kstride=fs),
                        ALU.mult,
                    )

        def w_update(ranges, hi_t):
            for (r0, r1), pair in ranges:
                if pair is not None:
                    wv = blockpair(W, r0, r1, pair)
                    pv = blockpair(P, r0, r1, pair)
                else:
                    wv, pv = wf(W, r0, r1), wf(P, r0, r1)
                nc.vector.tensor_tensor(wv, wv, pv, ALU.add)
                nc.vector.tensor_scalar(wv, wv, hi_t, 0.0, ALU.min, ALU.max)

        RANGES_HF = (((0, 3), 5), ((10, 14), None), ((22, 24), None))

        # ---- init ----
        nc.vector.memset(FA[:], 0.0)
        nc.vector.memset(FB[:], 0.0)
        nc.vector.memset(PVX2[0][:], 0.0)
        nc.vector.memset(PVX2[1][:], 0.0)
        nc.vector.memset(V[:], REST_V)
        nc.vector.memset(RST[:], RESET_V)
        nc.vector.memset(B30[:], -0.5 * MIDPOINT)
        nc.sync.dma_start(W[:], w0_d[:])
        nc.sync.dma_start(SM[:], sm_d[:])
        nc.sync.dma_start(MSKB[:], msk_d[:])
        XV = XINP.rearrange("p (t f) -> p t f", t=nsteps)
        xin_v = xin_d.rearrange("p (t f) -> p t f", t=nsteps)
        nc.sync.dma_start(XV[:, 0:1], xin_v[:, 0:1])
        nc.sync.dma_start(XV[:, 1:], xin_v[:, 1:])

        pid = nc.sync.partition_id()
        offL = nc.sync.snap((pid + NCORES - 1) % NCORES, min_val=0, max_val=NCORES - 1)
        offR = nc.sync.snap((pid + 1) % NCORES, min_val=0, max_val=NCORES - 1)

        fields = [FA, FB]
        for t in range(nsteps):
            FPREV = fields[t % 2]
            FOUT = fields[(t + 1) % 2]
            last = t == nsteps - 1
            S = SS[t % 2]
            pvx = PVX2[t % 2]       # this step's stdp reads pvx
            pvx_next = PVX2[(t + 1) % 2]

            # ---- syn: halo-dependent products (HF ones were emitted in the
            # previous iteration's overlap window), PE k-reduction ----
            if t > 0:
                syn_mults(FPREV, GROUPS_H)
                order = MM_ORDER_HF + MM_ORDER_H
                for i, k in enumerate(order):
                    nc.tensor.matmul(
                        TDP[:], smv[:, SM_SLICE[SLOTS[k][0]]], Pk[:, k],
                        start=(i == 0), stop=(i == len(order) - 1),
                    )
                s_t = float(np.float32(ONE_MINUS_DECAY * c[t - 1]))
                nc.vector.scalar_tensor_tensor(
                    SYN[:], TDP[:], s_t, XV[:, t], ALU.mult, ALU.add
                )
                nc.vector.scalar_tensor_tensor(
                    V[:], V[:], DECAY, SYN[:], ALU.mult, ALU.add
                )
            else:
                nc.vector.scalar_tensor_tensor(
                    V[:], V[:], DECAY, XV[:, t], ALU.mult, ALU.add
                )

            # ---- neuron update ----
            nc.vector.tensor_scalar(S[:], V[:], EXC_THR, None, ALU.is_ge)
            nc.sync.dma_start(spk_d[t], S[:])
            if last:
                continue
            nc.vector.tensor_scalar(SI[:], V[:], EXC_THR, None, ALU.is_ge)
            nc.vector.tensor_scalar(II[:], V[:], INH_THR, None, ALU.is_le)
            nc.scalar.activation(G[:], V[:], AT.Sigmoid, bias=B30[:, 0:1], scale=0.5)
            nc.vector.tensor_tensor(E[:], S[:], II[:], ALU.subtract)
            nc.vector.tensor_tensor(E[:], G[:], E[:], ALU.add)
            # out = clip01(...): boundary-strip rows first so the AllGather
            # input DMAs launch before the interior is written
            nc.vector.tensor_scalar(
                fint(FOUT)[0:NX, 0:2], c3(E)[0:NX, 0:2], 1.0, 0.0, ALU.min, ALU.max
            )
            nc.vector.tensor_scalar(
                fint(FOUT)[NX:P128, 2:4], c3(E)[NX:P128, 2:4],
                1.0, 0.0, ALU.min, ALU.max,
            )
            # ---- boundary strips -> AllGather (unmasked; edge-core wrap
            # garbage is neutralized by W'=0 on syn and the masked QF-halo
            # scale on STDP) ----
            agin = dr.tile([P128, 2 * NZ], DT, name=f"agin{t}")
            agout = dr.tile(
                [NCORES * P128, 2 * NZ], DT, addr_space="Shared", name=f"agout{t}"
            )
            agv = agin.rearrange("p (r z) -> p r z", z=NZ)
            nc.sync.dma_start(agv[0:NX], f3(FOUT)[0:NX, 2:4, 2 : 2 + NZ])
            nc.sync.dma_start(agv[NX:P128], f3(FOUT)[NX:P128, 4:6, 2 : 2 + NZ])
            nc.vector.tensor_scalar(
                fint(FOUT)[0:NX, 2:4], c3(E)[0:NX, 2:4], 1.0, 0.0, ALU.min, ALU.max
            )
            nc.vector.tensor_scalar(
                fint(FOUT)[NX:P128, 0:2], c3(E)[NX:P128, 0:2],
                1.0, 0.0, ALU.min, ALU.max,
            )
            nc.vector.copy_predicated(V[:], SI[:], RST[:])
            nc.gpsimd.collective_compute(
                "AllGather",
                ALU.bypass,
                replica_groups=[list(range(NCORES))],
                ins=[agin.opt()],
                outs=[agout.opt()],
            )
            nc.sync.dma_start(f3(FOUT)[0:NX, 6:8, 2 : 2 + NZ],
                              f3(FOUT)[NX:P128, 2:4, 2 : 2 + NZ])
            nc.sync.dma_start(f3(FOUT)[NX:P128, 0:2, 2 : 2 + NZ],
                              f3(FOUT)[0:NX, 4:6, 2 : 2 + NZ])

            # ---- overlap window: halo-free STDP + next-step prefetches ----
            do_stdp = t > 0
            if do_stdp:
                a_t = float(np.float32((ETA_LTP + ETA_LTD) / c[t]))
                b_t = float(np.float32(-ETA_LTD / c[t]))
                hi_t = float(np.float32(1.0 / c[t]))
                qf3 = f3(QF)
                fo3 = f3(FOUT)
                nc.vector.tensor_scalar(
                    qf3[:, 2:6], fo3[:, 2:6], a_t, b_t, ALU.mult, ALU.add
                )
                # per-step masked QF-halo scales (zero invalid halo sources)
                nc.vector.tensor_scalar(SA[:], MSKB[:], a_t, None, ALU.mult)
                stdp_mults(FPREV, pvx, GROUPS_HF)
                w_update(RANGES_HF, hi_t)

            # prefetch x-shifted prev chunks for next step's stdp
            if t + 1 < nsteps - 1:
                pvs = pvx_next.rearrange("p (s f) -> p s f", s=4)
                for dxv, sl in PVX_SLICE.items():
                    xshift_dma(nc.scalar, pvs[:, sl], fint(FOUT), -dxv)

            # next step's halo-free syn products (fills the collective wait)
            if t + 1 < nsteps:
                syn_mults(FOUT, GROUPS_HF)

            # ---- halo in from neighbors ----
            agf = agout.rearrange("p (r z) -> p r z", z=NZ)
            nc.sync.dma_start(
                f3(FOUT)[0:NX, 0:2, 2 : 2 + NZ],
                agf[bass.ds(offL * P128 + NX, NX)],
            )
            nc.sync.dma_start(
                f3(FOUT)[NX:P128, 6:8, 2 : 2 + NZ],
                agf[bass.ds(offR * P128, NX)],
            )

            # ---- halo-dependent STDP (interleaved mult/update for
            # per-range pipelining behind the two halo-in DMAs) ----
            if do_stdp:
                nc.vector.tensor_scalar(
                    qf3[:, 0:2], fo3[:, 0:2], SA[:, 0:1], b_t, ALU.mult, ALU.add
                )
                nc.vector.tensor_scalar(
                    qf3[:, 6:8], fo3[:, 6:8], SA[:, 1:2], b_t, ALU.mult, ALU.add
                )
                stdp_mults(FPREV, pvx, [("run", 14, 3, 1, 1, 0)])
                w_update((((14, 17), None),), hi_t)
                stdp_mults(FPREV, pvx, [("run", 17, 3, -1, 1, 0)])
                w_update((((17, 20), None),), hi_t)
                stdp_mults(FPREV, pvx,
                           [("pair", 3, 8, 1, 0, 0, "p01"),
                            ("pair", 4, 9, -1, 0, 0, "p01")])
                w_update((((3, 5), 8),), hi_t)
                stdp_mults(FPREV, pvx, [("pair", 20, 21, 2, 0, 4 * 68, 0)])
                w_update((((20, 22), None),), hi_t)

    nc.compile()
    return nc


def _shard_inputs(external_input, edge_values, edge_rows, edge_cols, nsteps):
    ext = np.ascontiguousarray(np.asarray(external_input, dtype=np.float32))[:nsteps]
    vals = np.asarray(edge_values, dtype=np.float32)
    rows = np.asarray(edge_rows, dtype=np.int64)
    cols = np.asarray(edge_cols, dtype=np.int64)

    dlin = cols - rows
    offs_lin = np.array([d[0] * NY * NZ + d[1] * NZ + d[2] for d in SLOTS])
    k_of = {int(v): i for i, v in enumerate(offs_lin)}
    ke = np.array([k_of[int(v)] for v in dlin], dtype=np.int64)
    Wd = np.zeros((NOFF, N), dtype=np.float32)
    Wd[ke, cols] = vals
    Wd = Wd.reshape(NOFF, NX, NY, NZ)

    Wsrc = np.zeros_like(Wd)
    for k, (dx, _, _) in enumerate(SLOTS):
        if dx >= 0:
            Wsrc[k, : NX - dx] = Wd[k, dx:]
        else:
            Wsrc[k, -dx:] = Wd[k, : NX + dx]

    ext4 = ext.reshape(nsteps, NX, NY, NZ) * np.float32(ONE_MINUS_DECAY)

    # shift matrices: slices (+1,-1,+2,-2,identity); SM_s[p,m]=1 iff m=p+dx
    sm = np.zeros((2 * NX, 5, 2 * NX), dtype=np.float32)
    for s, dxv in enumerate((1, -1, 2, -2, 0)):
        for h in (0, 1):
            for xs in range(NX):
                xm = xs + dxv
                if 0 <= xm < NX:
                    sm[h * NX + xs, s, h * NX + xm] = 1.0
    sm = sm.reshape(2 * NX, 5 * 2 * NX).astype(NPDT)

    in_maps = []
    for cidx in range(NCORES):
        ylo = cidx * YS
        sub = Wsrc[:, :, ylo : ylo + YS, :]
        tr = sub.transpose(1, 0, 2, 3)
        wc = np.concatenate(
            [tr[:, :, :YH, :].reshape(NX, NOFF * CH),
             tr[:, :, YH:, :].reshape(NX, NOFF * CH)], axis=0
        ).astype(NPDT)
        esub = ext4[:, :, ylo : ylo + YS, :].transpose(1, 0, 2, 3)
        xc = np.concatenate(
            [esub[:, :, :YH, :].reshape(NX, nsteps * CH),
             esub[:, :, YH:, :].reshape(NX, nsteps * CH)], axis=0
        ).astype(np.float32)
        # halo-validity masks: col 0 gates field rows 0:2 (h0 <- left
        # neighbor, h1 <- intra); col 1 gates rows 6:8 (h0 <- intra,
        # h1 <- right neighbor)
        msk = np.ones((2 * NX, 2), dtype=np.float32)
        if cidx == 0:
            msk[:NX, 0] = 0.0
        if cidx == NCORES - 1:
            msk[NX:, 1] = 0.0
        in_maps.append(
            {"w0": np.ascontiguousarray(wc), "xin": np.ascontiguousarray(xc),
             "msk": msk, "sm": sm}
        )
    return in_maps


def kernel(external_input, edge_values, edge_rows, edge_cols, num_steps):
    nsteps = int(num_steps)
    if nsteps not in _CACHE:
        _CACHE[nsteps] = _build_graph(nsteps)
    nc = _CACHE[nsteps]

    in_maps = _shard_inputs(external_input, edge_values, edge_rows, edge_cols, nsteps)
    res = bass_utils.run_bass_kernel_spmd(
        nc,
        in_maps,
        core_ids=list(range(NCORES)),
        trace=bool(int(os.environ.get("BRAIN_TRACE", "0"))),
    )

    out = np.empty((nsteps, NX, NY, NZ), dtype=np.float32)
    for cidx in range(NCORES):
        ylo = cidx * YS
        spk = res.results[cidx]["spk"].reshape(nsteps, 2, NX, YH, NZ)
        out[:, :, ylo : ylo + YH, :] = spk[:, 0]
        out[:, :, ylo + YH : ylo + YS, :] = spk[:, 1]
    kernel.last_results = res
    return out.reshape(nsteps, N)
